# revision 23
# baseline (speedup 1.0000x reference)
"""BigBird encoder + vocab projection on 8 Trainium2 NeuronCores.

Sequence-sharded transformer (core c owns rows [256c, 256c+256) = 4 query
blocks), per-layer split AllGather of K^T then V (overlapped with Q proj +
attention), vocab-sharded final projection.  Activations live transposed
(xT [768part, 256free]) so every matmul consumes weights as stored.
BigBird sparsity (window + global + random + dedup + key_mask) is folded
into a per-core 0/1 mask multiplied into exp(scores) — mathematically
identical to the reference's gather+softmax.  bf16 matmuls, fp32
accumulate/residual.

Perf structure (v2): K-AG and V-AG launched as soon as each projection
finishes; copy-outs are two large p-major DMAs each; score matmuls of head
pairs interleave on PE row-groups 0-63/64-127 for 2x concurrency; softmax
reciprocals batched into one [12,SQ] op; LayerNorm stats accumulate
chunk-wise inside the residual loops to keep PE busy (HAM warm).
"""
import os, sys
os.environ.setdefault("JAX_PLATFORMS", "")
import numpy as np
import ml_dtypes

sys.path.insert(0, "/opt/trn_rl_repo")

import concourse.bass as bass
import concourse.tile as tile
from concourse import bacc, mybir
from concourse.bass_utils import run_bass_kernel_spmd

BF16 = mybir.dt.bfloat16
F32 = mybir.dt.float32
AF = mybir.ActivationFunctionType
MUL = mybir.AluOpType.mult
ADD = mybir.AluOpType.add
SUB = mybir.AluOpType.subtract

B, S, D, F, V = 1, 2048, 768, 3072, 50358
L, H, HD, BS, NB, R = 12, 12, 64, 64, 32, 3
NC = 8
SQ = S // NC                # 256
DC = D // 128               # 6
FC = F // 128               # 24
KC = S // 128               # 16
VSH = 6400                  # padded vocab shard (50 x 128)
VN_E = HD + 1               # 65 cols per head in V-normal (ones col for rowsum)
VNF = H * VN_E              # 780 flat per (p, rc)
VN_FLAT = 128 * 2 * VNF     # 199680
KT_FLAT = 128 * DC * SQ     # 196608
XF_FLAT = 128 * DC * SQ

_nc_cache = {}


def _block_map(nb, r, seed=0):
    rng = np.random.default_rng(seed)
    idx = np.zeros((nb, 5 + r), np.int32)
    for i in range(nb):
        lst = [0, nb - 1, max(i - 1, 0), i, min(i + 1, nb - 1)]
        cand = np.setdiff1d(np.arange(nb), np.array(lst))
        lst += list(rng.choice(cand, r, replace=False))
        for j, b in enumerate(lst):
            idx[i, j] = int(b)
    return idx


def build():
    nc = bacc.Bacc("TRN2", target_bir_lowering=False, debug=False, num_devices=NC)
    ET = nc.dram_tensor("e_t", [128, DC, SQ], F32, kind="ExternalInput")
    WQ = nc.dram_tensor("wq", [L, 128, DC, D], BF16, kind="ExternalInput")
    WK = nc.dram_tensor("wk", [L, 128, DC, D], BF16, kind="ExternalInput")
    WV = nc.dram_tensor("wv", [L, 128, DC, D], BF16, kind="ExternalInput")
    WO = nc.dram_tensor("wo", [L, 128, DC, D], BF16, kind="ExternalInput")
    W1 = nc.dram_tensor("w1", [L, 128, DC, F], BF16, kind="ExternalInput")
    W2 = nc.dram_tensor("w2", [L, 128, FC, D], BF16, kind="ExternalInput")
    BQ = nc.dram_tensor("bq_t", [L, 128, DC], F32, kind="ExternalInput")  # x0.125
    BK = nc.dram_tensor("bk_t", [L, 128, DC], F32, kind="ExternalInput")
    B1 = nc.dram_tensor("b1_t", [L, 128, FC], F32, kind="ExternalInput")
    BROW = nc.dram_tensor("brow", [L, 3, D], BF16, kind="ExternalInput")  # bv,bo,b2
    LNS = nc.dram_tensor("ln_s", [128, 2 * L + 1, DC], F32, kind="ExternalInput")
    LNB = nc.dram_tensor("ln_b", [128, 2 * L + 1, DC], F32, kind="ExternalInput")
    M01 = nc.dram_tensor("m01t", [128, KC, SQ], BF16, kind="ExternalInput")
    ONEH = nc.dram_tensor("oneh", [128, 4, HD], BF16, kind="ExternalInput")
    FCW = nc.dram_tensor("fcw", [128, DC, VSH], BF16, kind="ExternalInput")
    FCB = nc.dram_tensor("fcb", [1, VSH], BF16, kind="ExternalInput")
    OUT = nc.dram_tensor("out_t", [VSH // 128, 128, S], F32, kind="ExternalOutput")

    with tile.TileContext(nc) as tc:
        with tc.tile_pool(name="dram", bufs=1, space="DRAM") as dram, \
             tc.tile_pool(name="res", bufs=1) as res, \
             tc.tile_pool(name="const", bufs=1) as const:
            bnck_ins = [dram.tile([KT_FLAT], BF16, tag=f"cki{l}", name=f"cki{l}")
                        for l in range(L)]
            bnck_outs = [dram.tile([NC * KT_FLAT], BF16, addr_space="Shared",
                                   tag=f"cko{l}", name=f"cko{l}")
                         for l in range(L)]
            bncv_ins = [dram.tile([VN_FLAT], BF16, tag=f"cvi{l}", name=f"cvi{l}")
                        for l in range(L)]
            bncv_outs = [dram.tile([NC * VN_FLAT], BF16, addr_space="Shared",
                                   tag=f"cvo{l}", name=f"cvo{l}")
                         for l in range(L)]
            bncx_in = dram.tile([XF_FLAT], BF16)
            bncx_out = dram.tile([NC * XF_FLAT], BF16, addr_space="Shared")

            xT = res.tile([128, DC, SQ], F32)
            nc.sync.dma_start(xT[:], ET[:])
            m01 = const.tile([128, KC, SQ], BF16)
            nc.sync.dma_start(m01[:], M01[:])
            oneh = const.tile([128, 4, HD], BF16)
            nc.sync.dma_start(oneh[:], ONEH[:])
            lns = const.tile([128, 2 * L + 1, DC], F32)
            lnb = const.tile([128, 2 * L + 1, DC], F32)
            nc.sync.dma_start(lns[:], LNS[:])
            nc.sync.dma_start(lnb[:], LNB[:])
            ones_b = const.tile([128, 1], BF16)
            nc.vector.memset(ones_b[:], 1.0)
            ones_fr = const.tile([1, 128], F32)
            nc.vector.memset(ones_fr[:], 1.0)
            onesrow = const.tile([1, 512], BF16)
            nc.vector.memset(onesrow[:], 1.0)
            eps = const.tile([1, 1], F32)
            nc.vector.memset(eps[:], 1e-12)

            with tc.tile_pool(name="wp", bufs=1) as wp, \
                 tc.tile_pool(name="wf", bufs=2) as wf, \
                 tc.tile_pool(name="act", bufs=2) as act, \
                 tc.tile_pool(name="ag", bufs=1) as ag, \
                 tc.tile_pool(name="sm", bufs=2) as sm, \
                 tc.tile_pool(name="ps", bufs=2, space="PSUM") as ps, \
                 tc.tile_pool(name="pssc", bufs=2, space="PSUM") as pssc, \
                 tc.tile_pool(name="ps1", bufs=2, space="PSUM") as ps1:

                xbf = act.tile([128, DC, SQ], BF16, tag="xbf", bufs=1)
                # softmax denominator parking (head h -> partition 32*(h%4),
                # group h//4); memset once so full-tile reciprocal never sees
                # uninitialized bits
                dnt = sm.tile([128, 3, SQ], F32, tag="dnt", bufs=1)
                nc.vector.memset(dnt[:], 1.0)

                # ---- layernorm helpers (chunk-pipelined) ----
                def ln_stats_start():
                    su = pssc.tile([1, SQ], F32, tag="sc", name="lnsum")
                    sq = pssc.tile([1, SQ], F32, tag="sc", name="lnssq")
                    return su, sq

                def ln_stats_chunk(stats, oc, src_bf):
                    su, sq = stats
                    sqs = act.tile([128, SQ], BF16, tag="sqs")
                    nc.scalar.activation(sqs[:], src_bf, AF.Square)
                    nc.tensor.matmul(su[:], ones_b[:], src_bf,
                                     start=(oc == 0), stop=(oc == DC - 1))
                    nc.tensor.matmul(sq[:], ones_b[:], sqs[:],
                                     start=(oc == 0), stop=(oc == DC - 1))

                def ln_tail_apply(stats, li):
                    su, sq = stats
                    nmean = sm.tile([1, SQ], F32, tag="nmean")
                    ab = sm.tile([1, 2 * SQ], F32, tag="ab")
                    nc.vector.tensor_scalar_mul(nmean[:], su[:], -1.0 / D)
                    nc.vector.tensor_scalar_mul(ab[:, 0:SQ], sq[:], 1.0 / D)
                    t0 = sm.tile([1, SQ], F32, tag="lnt0")
                    nc.vector.tensor_tensor(t0[:], nmean[:], nmean[:], op=MUL)
                    nc.vector.tensor_tensor(ab[:, 0:SQ], ab[:, 0:SQ], t0[:], op=SUB)
                    # rstd = 1/sqrt(var + eps)
                    nc.scalar.activation(t0[:], ab[:, 0:SQ], AF.Sqrt, bias=eps[:])
                    nc.vector.reciprocal_approx_fast(ab[:, 0:SQ], t0[:])
                    nc.vector.tensor_tensor(ab[:, SQ:], nmean[:], ab[:, 0:SQ],
                                            op=MUL)              # -mean*rstd
                    bc = pssc.tile([128, 2 * SQ], F32, tag="sc", name="lnbc")
                    nc.tensor.matmul(bc[:], ones_fr[:], ab[:], start=True, stop=True)
                    # xbf (bf16, consumed by matmuls) is produced with the
                    # shortest possible vector chain; the fp32 xT update (+b)
                    # is deferred so it doesn't gate the next phase's PE work
                    t2s = []
                    for oc in range(DC):
                        t1 = sm.tile([128, SQ], F32, tag="lnt1")
                        nc.vector.scalar_tensor_tensor(
                            t1[:], xT[:, oc, :], lns[:, li, oc:oc + 1],
                            bc[:, 0:SQ], op0=MUL, op1=MUL)
                        t2 = sm.tile([128, SQ], F32, tag="lnt2", bufs=6)
                        nc.vector.scalar_tensor_tensor(
                            t2[:], bc[:, SQ:2 * SQ], lns[:, li, oc:oc + 1],
                            t1[:], op0=MUL, op1=ADD)
                        nc.scalar.activation(xbf[:, oc, :], t2[:], AF.Identity,
                                             bias=lnb[:, li, oc:oc + 1])
                        t2s.append(t2)
                    for oc in range(DC):
                        nc.vector.tensor_scalar_add(xT[:, oc, :], t2s[oc][:],
                                                    lnb[:, li, oc:oc + 1])

                # ---- embedding layernorm (standalone) ----
                wk = wp.tile([128, DC, D], BF16, tag="wk")
                nc.sync.dma_start(wk[:], WK[0])
                wv = wp.tile([128, DC, D], BF16, tag="wv")
                nc.sync.dma_start(wv[:], WV[0])
                wq = wp.tile([128, DC, D], BF16, tag="wq")
                nc.sync.dma_start(wq[:], WQ[0])
                bq = wp.tile([128, DC], F32, tag="bq")
                nc.sync.dma_start(bq[:], BQ[0])
                bk = wp.tile([128, DC], F32, tag="bk")
                nc.sync.dma_start(bk[:], BK[0])
                bvr = wp.tile([1, D], BF16, tag="bvr")
                nc.sync.dma_start(bvr[:], BROW[0][0:1, :])
                bor = wp.tile([1, D], BF16, tag="bor")
                nc.sync.dma_start(bor[:], BROW[0][1:2, :])
                b2r = wp.tile([1, D], BF16, tag="b2r")
                nc.sync.dma_start(b2r[:], BROW[0][2:3, :])
                wo = wp.tile([128, DC, D], BF16, tag="wo")
                nc.sync.dma_start(wo[:], WO[0])

                st0 = ln_stats_start()
                for oc in range(DC):
                    xs = act.tile([128, SQ], BF16, tag="xs")
                    nc.vector.tensor_copy(xs[:], xT[:, oc, :])
                    ln_stats_chunk(st0, oc, xs[:])
                ln_tail_apply(st0, 0)

                for l in range(L):
                    with nc.named_scope(f"L{l}"):
                        # ---- K^T projection + AG launch ----
                        ktb = act.tile([128, DC, SQ], BF16, tag="ktb", bufs=1)
                        for oc in range(DC):
                            kp = ps.tile([128, SQ], F32, tag="proj")
                            for dc in range(DC):
                                nc.tensor.matmul(
                                    kp[:], wk[:, dc, 128 * oc:128 * (oc + 1)],
                                    xbf[:, dc, :], start=(dc == 0),
                                    stop=(dc == DC - 1))
                            nc.scalar.activation(ktb[:, oc, :], kp[:],
                                                 AF.Identity,
                                                 bias=bk[:, oc:oc + 1])
                        nc.sync.dma_start(
                            bnck_ins[l][:].rearrange("(p i q) -> p i q",
                                                     p=128, i=DC), ktb[:])
                        nc.gpsimd.collective_compute(
                            "AllGather", mybir.AluOpType.bypass,
                            replica_groups=[list(range(NC))],
                            ins=[bnck_ins[l][:].opt()],
                            outs=[bnck_outs[l][:].opt()])

                        # ---- V normal + AG launch ----
                        vnb = act.tile([128, 2, VNF], BF16, tag="vnb", bufs=1)
                        nc.vector.memset(
                            vnb[:].rearrange("p rc (h e) -> p rc h e",
                                             h=H)[:, :, :, HD:], 1.0)
                        for rc in range(2):
                            for hf in range(2):
                                cs = slice(384 * hf, 384 * (hf + 1))
                                vp = ps.tile([128, 384], F32, tag="proj",
                                             name="vproj")
                                for dc in range(DC):
                                    nc.tensor.matmul(
                                        vp[:], xbf[:, dc, 128 * rc:128 * (rc + 1)],
                                        wv[:, dc, cs], start=(dc == 0), stop=False)
                                nc.tensor.matmul(vp[:], onesrow[:1, :128],
                                                 bvr[:1, cs], start=False, stop=True)
                                nc.vector.tensor_copy(
                                    vnb[:, rc, 390 * hf:390 * (hf + 1)].rearrange(
                                        "p (h e) -> p h e", h=6)[:, :, 0:HD],
                                    vp[:].rearrange("p (h e) -> p h e", h=6))
                        nc.sync.dma_start(
                            bncv_ins[l][:].rearrange("(p rc f) -> p rc f",
                                                     p=128, rc=2), vnb[:])
                        nc.gpsimd.collective_compute(
                            "AllGather", mybir.AluOpType.bypass,
                            replica_groups=[list(range(NC))],
                            ins=[bncv_ins[l][:].opt()],
                            outs=[bncv_outs[l][:].opt()])

                        # ---- Q^T (overlaps AG) ----
                        qtb = act.tile([128, DC, SQ], BF16, tag="qtb", bufs=1)
                        for oc in range(DC):
                            qp = ps.tile([128, SQ], F32, tag="proj")
                            for dc in range(DC):
                                nc.tensor.matmul(
                                    qp[:], wq[:, dc, 128 * oc:128 * (oc + 1)],
                                    xbf[:, dc, :], start=(dc == 0),
                                    stop=(dc == DC - 1))
                            nc.scalar.activation(qtb[:, oc, :], qp[:],
                                                 AF.Identity,
                                                 bias=bq[:, oc:oc + 1],
                                                 scale=0.125)

                        # prefetch next layer's QKV weights
                        if l + 1 < L:
                            wk = wp.tile([128, DC, D], BF16, tag="wk")
                            nc.sync.dma_start(wk[:], WK[l + 1])
                            wv = wp.tile([128, DC, D], BF16, tag="wv")
                            nc.sync.dma_start(wv[:], WV[l + 1])
                            wq = wp.tile([128, DC, D], BF16, tag="wq")
                            nc.sync.dma_start(wq[:], WQ[l + 1])
                            bq = wp.tile([128, DC], F32, tag="bq")
                            nc.sync.dma_start(bq[:], BQ[l + 1])
                            bk = wp.tile([128, DC], F32, tag="bk")
                            nc.sync.dma_start(bk[:], BK[l + 1])

                        # ---- AG copy-outs (2 halves each, gpsimd queue) ----
                        ktag = ag.tile([128, NC * DC, SQ], BF16, tag="ktag")
                        vnag = ag.tile([128, KC, VNF], BF16, tag="vnag")
                        for hh in range(2):
                            o = 4 * hh * KT_FLAT
                            nc.gpsimd.dma_start(
                                ktag[:, 24 * hh:24 * (hh + 1), :].rearrange(
                                    "p (r i) q -> p r i q", r=4),
                                bnck_outs[l][o:o + 4 * KT_FLAT].rearrange(
                                    "(r p i q) -> p r i q", r=4, p=128, i=DC))
                        for hh in range(2):
                            o = 4 * hh * VN_FLAT
                            nc.gpsimd.dma_start(
                                vnag[:, 8 * hh:8 * (hh + 1), :].rearrange(
                                    "p (r rc) f -> p r rc f", r=4),
                                bncv_outs[l][o:o + 4 * VN_FLAT].rearrange(
                                    "(r p rc f) -> p r rc f", r=4, p=128, rc=2))

                        # ---- attention: head pairs, interleaved row groups ----
                        ctxr = act.tile([128, DC, SQ], BF16, tag="ctxr", bufs=1)
                        ctxb = act.tile([128, DC, SQ], BF16, tag="ctxb", bufs=1)
                        for t in range(6):
                            hA, hB = 2 * t, 2 * t + 1
                            ctA = ps1.tile([VN_E, SQ], F32, tag="ctx")
                            ctB = ps1.tile([VN_E, SQ], F32, tag="ctx")
                            for w in range(4):
                                scA = pssc.tile([128, 4, SQ], F32, tag="sc")
                                scB = pssc.tile([128, 4, SQ], F32, tag="sc")
                                for j in range(4):
                                    kc = 4 * w + j
                                    r, hf = kc // 2, kc % 2
                                    nc.tensor.matmul(
                                        scA[:, j, :],
                                        ktag[0:64, DC * r + t,
                                             128 * hf:128 * (hf + 1)],
                                        qtb[0:64, t, :], start=True, stop=True)
                                    nc.tensor.matmul(
                                        scB[:, j, :],
                                        ktag[64:128, DC * r + t,
                                             128 * hf:128 * (hf + 1)],
                                        qtb[64:128, t, :], start=True, stop=True)
                                prA = sm.tile([128, 4, SQ], BF16, tag="pr",
                                              bufs=4)
                                prB = sm.tile([128, 4, SQ], BF16, tag="pr",
                                              bufs=4)
                                nc.scalar.activation(prA[:], scA[:], AF.Exp)
                                nc.scalar.activation(prB[:], scB[:], AF.Exp)
                                nc.vector.tensor_tensor(
                                    prA[:], prA[:], m01[:, 4 * w:4 * w + 4, :],
                                    op=MUL)
                                nc.vector.tensor_tensor(
                                    prB[:], prB[:], m01[:, 4 * w:4 * w + 4, :],
                                    op=MUL)
                                for j in range(4):
                                    kc = 4 * w + j
                                    nc.tensor.matmul(
                                        ctA[:], vnag[:, kc, 65 * hA:65 * hA + 65],
                                        prA[:, j, :], start=(kc == 0),
                                        stop=(kc == KC - 1))
                                    nc.tensor.matmul(
                                        ctB[:], vnag[:, kc, 65 * hB:65 * hB + 65],
                                        prB[:, j, :], start=(kc == 0),
                                        stop=(kc == KC - 1))
                            pA, gA = 32 * (hA % 4), hA // 4
                            pB, gB = 32 * (hB % 4), hB // 4
                            nc.vector.tensor_copy(dnt[pA:pA + 1, gA, :],
                                                  ctA[HD:HD + 1, :])
                            nc.vector.tensor_copy(dnt[pB:pB + 1, gB, :],
                                                  ctB[HD:HD + 1, :])
                            nc.vector.tensor_copy(ctxr[0:64, t, :], ctA[0:HD, :])
                            nc.vector.tensor_copy(ctxr[64:128, t, :], ctB[0:HD, :])
                        rcpf = sm.tile([128, 3, SQ], F32, tag="rcpf", bufs=1)
                        nc.vector.reciprocal_approx_fast(rcpf[:], dnt[:])
                        rcp = sm.tile([128, 3, SQ], BF16, tag="rcp", bufs=1)
                        nc.vector.tensor_copy(rcp[:], rcpf[:])
                        for h in range(H):
                            ki, kp_ = h // 2, 64 * (h % 2)
                            rbc = ps.tile([64, SQ], F32, tag="proj", name="rbc")
                            nc.tensor.matmul(rbc[:], oneh[:, h % 4, :],
                                             rcp[:, h // 4, :],
                                             start=True, stop=True)
                            nc.vector.tensor_tensor(ctxb[kp_:kp_ + 64, ki, :],
                                                    ctxr[kp_:kp_ + 64, ki, :],
                                                    rbc[:], op=MUL)

                        # ---- Wo + residual + LN1 stats (chunked) ----
                        st1 = ln_stats_start()
                        for oc in range(DC):
                            op_ = ps.tile([128, SQ], F32, tag="proj")
                            for dc in range(DC):
                                nc.tensor.matmul(
                                    op_[:], wo[:, dc, 128 * oc:128 * (oc + 1)],
                                    ctxb[:, dc, :], start=(dc == 0), stop=False)
                            nc.tensor.matmul(op_[:],
                                             bor[:1, 128 * oc:128 * (oc + 1)],
                                             onesrow[:1, :SQ], start=False,
                                             stop=True)
                            xs = act.tile([128, SQ], BF16, tag="xs")
                            nc.vector.scalar_tensor_tensor(
                                xs[:], xT[:, oc, :], 1.0, op_[:],
                                op0=MUL, op1=ADD)
                            nc.vector.tensor_tensor(xT[:, oc, :], xT[:, oc, :],
                                                    op_[:], op=ADD)
                            ln_stats_chunk(st1, oc, xs[:])
                        ln_tail_apply(st1, 1 + 2 * l)

                        # ---- FFN ----
                        b1 = wp.tile([128, FC], F32, tag="b1")
                        nc.sync.dma_start(b1[:], B1[l])
                        htb = act.tile([128, FC, SQ], BF16, tag="htb", bufs=1)
                        for qi in range(6):
                            w1q = wf.tile([128, DC, F // 6], BF16, tag="w1q")
                            nc.sync.dma_start(
                                w1q[:], W1[l][:, :, (F // 6) * qi:(F // 6) * (qi + 1)])
                            for oc6 in range(4):
                                oc = 4 * qi + oc6
                                hp = ps.tile([128, SQ], F32, tag="proj")
                                for dc in range(DC):
                                    nc.tensor.matmul(
                                        hp[:], w1q[:, dc, 128 * oc6:128 * (oc6 + 1)],
                                        xbf[:, dc, :], start=(dc == 0),
                                        stop=(dc == DC - 1))
                                nc.scalar.activation(htb[:, oc, :], hp[:],
                                                     AF.Gelu_apprx_tanh,
                                                     bias=b1[:, oc:oc + 1])
                        st2 = ln_stats_start()
                        for hi in range(3):
                            w2h = wf.tile([128, FC, D // 3], BF16, tag="w2h")
                            nc.sync.dma_start(
                                w2h[:], W2[l][:, :, (D // 3) * hi:(D // 3) * (hi + 1)])
                            for oc3 in range(2):
                                oc = 2 * hi + oc3
                                yp = ps.tile([128, SQ], F32, tag="proj")
                                for kc in range(FC):
                                    nc.tensor.matmul(
                                        yp[:], w2h[:, kc, 128 * oc3:128 * (oc3 + 1)],
                                        htb[:, kc, :], start=(kc == 0), stop=False)
                                nc.tensor.matmul(
                                    yp[:], b2r[:1, 128 * oc:128 * (oc + 1)],
                                    onesrow[:1, :SQ], start=False, stop=True)
                                xs = act.tile([128, SQ], BF16, tag="xs")
                                nc.vector.scalar_tensor_tensor(
                                    xs[:], xT[:, oc, :], 1.0, yp[:],
                                    op0=MUL, op1=ADD)
                                nc.vector.tensor_tensor(xT[:, oc, :], xT[:, oc, :],
                                                        yp[:], op=ADD)
                                ln_stats_chunk(st2, oc, xs[:])
                        ln_tail_apply(st2, 2 + 2 * l)
                        if l + 1 < L:
                            bvr = wp.tile([1, D], BF16, tag="bvr")
                            nc.sync.dma_start(bvr[:], BROW[l + 1][0:1, :])
                            bor = wp.tile([1, D], BF16, tag="bor")
                            nc.sync.dma_start(bor[:], BROW[l + 1][1:2, :])
                            b2r = wp.tile([1, D], BF16, tag="b2r")
                            nc.sync.dma_start(b2r[:], BROW[l + 1][2:3, :])
                            wo = wp.tile([128, DC, D], BF16, tag="wo")
                            nc.sync.dma_start(wo[:], WO[l + 1])

            with tc.tile_pool(name="fin", bufs=1) as fin, \
                 tc.tile_pool(name="fw", bufs=4) as fw, \
                 tc.tile_pool(name="fo", bufs=3) as fo, \
                 tc.tile_pool(name="fps", bufs=2, space="PSUM") as fps:
                xbf2 = fin.tile([128, DC, SQ], BF16)
                nc.vector.tensor_copy(xbf2[:], xT[:])
                nc.sync.dma_start(
                    bncx_in[:].rearrange("(p i q) -> p i q", p=128, i=DC), xbf2[:])
                nc.gpsimd.collective_compute(
                    "AllGather", mybir.AluOpType.bypass,
                    replica_groups=[list(range(NC))],
                    ins=[bncx_in[:].opt()], outs=[bncx_out[:].opt()])
                xf = fin.tile([128, DC, S], BF16)
                nc.gpsimd.dma_start(
                    xf[:].rearrange("p i (r q) -> p i r q", r=NC),
                    bncx_out[:].rearrange("(r p i q) -> p i r q",
                                          r=NC, p=128, i=DC))
                fcb = fin.tile([1, VSH], BF16)
                nc.sync.dma_start(fcb[:], FCB[:])
                for oc in range(VSH // 128):
                    fwt = fw.tile([128, DC, 128], BF16, tag="fwt")
                    nc.sync.dma_start(fwt[:], FCW[:, :, 128 * oc:128 * (oc + 1)])
                    fp = fps.tile([128, S], F32, tag="fp")
                    for ncol in range(4):
                        cs = slice(512 * ncol, 512 * (ncol + 1))
                        for dc in range(DC):
                            nc.tensor.matmul(fp[:, cs], fwt[:, dc, :],
                                             xf[:, dc, cs], start=(dc == 0),
                                             stop=False)
                        nc.tensor.matmul(fp[:, cs],
                                         fcb[0:1, 128 * oc:128 * (oc + 1)],
                                         onesrow[:1, :], start=False, stop=True)
                    ot = fo.tile([128, S], F32, tag="ot")
                    if oc % 2 == 0:
                        nc.scalar.activation(ot[:], fp[:], AF.Copy)
                    else:
                        nc.vector.tensor_copy(ot[:], fp[:])
                    nc.sync.dma_start(OUT[oc], ot[:])
    nc.finalize()
    return nc


def kernel(input_ids, attention_mask, token_type_ids, word_emb, pos_emb, type_emb,
           emb_ln_s, emb_ln_b, Wq, Wk, Wv, bq, bk, bv, Wo, bo, ln1_s, ln1_b,
           W1, b1, W2, b2, ln2_s, ln2_b, fc_w, fc_b):
    f32 = np.float32
    bf = ml_dtypes.bfloat16
    ids = np.asarray(input_ids)[0]
    tt = np.asarray(token_type_ids)[0]
    am = np.asarray(attention_mask)[0].astype(f32)
    E = (np.asarray(word_emb, f32)[ids] + np.asarray(pos_emb, f32)
         + np.asarray(type_emb, f32)[tt])

    def tp(w):  # [D, N] -> [128, DC, N] bf16
        return np.ascontiguousarray(
            np.asarray(w, f32).reshape(-1, 128, w.shape[-1]).transpose(1, 0, 2)
        ).astype(bf)

    def col(b, scale=1.0):  # [N*128] -> [128, N] f32
        b = np.asarray(b, f32) * scale
        return np.ascontiguousarray(b.reshape(-1, 128).T).astype(f32)

    wq_h = np.stack([tp(Wq[l]) for l in range(L)])
    wk_h = np.stack([tp(Wk[l]) for l in range(L)])
    wv_h = np.stack([tp(Wv[l]) for l in range(L)])
    wo_h = np.stack([tp(Wo[l]) for l in range(L)])
    w1_h = np.stack([tp(W1[l]) for l in range(L)])
    w2_h = np.stack([tp(W2[l]) for l in range(L)])
    bq_h = np.stack([col(bq[l], 0.125) for l in range(L)])
    bk_h = np.stack([col(bk[l]) for l in range(L)])
    b1_h = np.stack([col(b1[l]) for l in range(L)])
    brow_h = np.stack([np.stack([np.asarray(bv[l], f32), np.asarray(bo[l], f32),
                                 np.asarray(b2[l], f32)]) for l in range(L)]).astype(bf)

    lns_list = [np.asarray(emb_ln_s, f32)]
    lnb_list = [np.asarray(emb_ln_b, f32)]
    for l in range(L):
        lns_list += [np.asarray(ln1_s[l], f32), np.asarray(ln2_s[l], f32)]
        lnb_list += [np.asarray(ln1_b[l], f32), np.asarray(ln2_b[l], f32)]
    ln_s = np.stack([s.reshape(DC, 128).T for s in lns_list], axis=1)
    ln_b = np.stack([s.reshape(DC, 128).T for s in lnb_list], axis=1)

    blk = _block_map(NB, R)
    allowed = np.zeros((NB, NB), f32)
    for i in range(NB):
        allowed[i, blk[i]] = 1.0
    allowed[0, :] = 1.0
    allowed[NB - 1, :] = 1.0
    allow_key = np.repeat(allowed, BS, axis=1) * am[None, :]

    oneh_h = np.zeros((128, 4, HD), f32)
    for k in range(4):
        oneh_h[32 * k, k, :] = 1.0
    oneh_h = oneh_h.astype(bf)

    fcw_pad = np.zeros((D, VSH * NC), f32)
    fcw_pad[:, :V] = np.asarray(fc_w, f32)
    fcb_pad = np.zeros((VSH * NC,), f32)
    fcb_pad[:V] = np.asarray(fc_b, f32)

    if "k" not in _nc_cache:
        _nc_cache["k"] = build()
    nc = _nc_cache["k"]

    in_maps = []
    for c in range(NC):
        e_shard = E[SQ * c:SQ * (c + 1)].T.reshape(DC, 128, SQ).transpose(1, 0, 2)
        # flat mask [128, KC, SQ]: m01[p, kc, q] = allow(qblock(q), key kc*128+p)
        m01 = np.repeat(
            allow_key[4 * c:4 * c + 4].T.reshape(KC, 128, 4).transpose(1, 0, 2),
            BS, axis=2)
        in_maps.append({
            "e_t": np.ascontiguousarray(e_shard).astype(f32),
            "wq": wq_h, "wk": wk_h, "wv": wv_h, "wo": wo_h,
            "w1": w1_h, "w2": w2_h,
            "bq_t": bq_h, "bk_t": bk_h, "b1_t": b1_h, "brow": brow_h,
            "ln_s": ln_s, "ln_b": ln_b,
            "m01t": np.ascontiguousarray(m01).astype(bf),
            "oneh": oneh_h,
            "fcw": tp(fcw_pad[:, VSH * c:VSH * (c + 1)]),
            "fcb": fcb_pad[None, VSH * c:VSH * (c + 1)].astype(bf),
        })

    trace = bool(int(os.environ.get("BB_TRACE", "0")))
    res = run_bass_kernel_spmd(nc, in_maps, core_ids=list(range(NC)), trace=trace)
    kernel.last_exec_ns = getattr(res, "exec_time_ns", None)
    kernel.last_result = res
    outs = [res.results[c]["out_t"].reshape(VSH, S).T for c in range(NC)]
    logits = np.concatenate(outs, axis=1)[:, :V]
    return logits[None].astype(np.float32)


# revision 24
# speedup vs baseline: 1.0195x; 1.0195x over previous
"""BigBird encoder + vocab projection on 8 Trainium2 NeuronCores.

Sequence-sharded transformer (core c owns rows [256c, 256c+256) = 4 query
blocks), per-layer split AllGather of K^T then V (overlapped with Q proj +
attention), vocab-sharded final projection.  Activations live transposed
(xT [768part, 256free]) so every matmul consumes weights as stored.
BigBird sparsity (window + global + random + dedup + key_mask) is folded
into a per-core 0/1 mask multiplied into exp(scores) — mathematically
identical to the reference's gather+softmax.  bf16 matmuls, fp32
accumulate/residual.

Perf structure (v2): K-AG and V-AG launched as soon as each projection
finishes; copy-outs are two large p-major DMAs each; score matmuls of head
pairs interleave on PE row-groups 0-63/64-127 for 2x concurrency; softmax
reciprocals batched into one [12,SQ] op; LayerNorm stats accumulate
chunk-wise inside the residual loops to keep PE busy (HAM warm).
"""
import os, sys
os.environ.setdefault("JAX_PLATFORMS", "")
import numpy as np
import ml_dtypes

sys.path.insert(0, "/opt/trn_rl_repo")

import concourse.bass as bass
import concourse.tile as tile
from concourse import bacc, mybir
from concourse.bass_utils import run_bass_kernel_spmd

BF16 = mybir.dt.bfloat16
FP8 = mybir.dt.float8e4
F32 = mybir.dt.float32
AF = mybir.ActivationFunctionType
MUL = mybir.AluOpType.mult
ADD = mybir.AluOpType.add
SUB = mybir.AluOpType.subtract

B, S, D, F, V = 1, 2048, 768, 3072, 50358
L, H, HD, BS, NB, R = 12, 12, 64, 64, 32, 3
NC = 8
SQ = S // NC                # 256
DC = D // 128               # 6
FC = F // 128               # 24
KC = S // 128               # 16
VSH = 6400                  # padded vocab shard (50 x 128)
VN_E = HD + 1               # 65 cols per head in V-normal (ones col for rowsum)
VNF = H * VN_E              # 780 flat per (p, rc)
VN_FLAT = 128 * 2 * VNF     # 199680
KT_FLAT = 128 * DC * SQ     # 196608
XF_FLAT = 128 * DC * SQ

_nc_cache = {}


def _block_map(nb, r, seed=0):
    rng = np.random.default_rng(seed)
    idx = np.zeros((nb, 5 + r), np.int32)
    for i in range(nb):
        lst = [0, nb - 1, max(i - 1, 0), i, min(i + 1, nb - 1)]
        cand = np.setdiff1d(np.arange(nb), np.array(lst))
        lst += list(rng.choice(cand, r, replace=False))
        for j, b in enumerate(lst):
            idx[i, j] = int(b)
    return idx


def build():
    nc = bacc.Bacc("TRN2", target_bir_lowering=False, debug=False, num_devices=NC)
    ET = nc.dram_tensor("e_t", [128, DC, SQ], F32, kind="ExternalInput")
    WQ = nc.dram_tensor("wq", [L, 128, DC, D], BF16, kind="ExternalInput")
    WK = nc.dram_tensor("wk", [L, 128, DC, D], BF16, kind="ExternalInput")
    WV = nc.dram_tensor("wv", [L, 128, DC, D], BF16, kind="ExternalInput")
    WO = nc.dram_tensor("wo", [L, 128, DC, D], BF16, kind="ExternalInput")
    W1 = nc.dram_tensor("w1", [L, 128, DC, F], BF16, kind="ExternalInput")
    W2 = nc.dram_tensor("w2", [L, 128, FC, D], BF16, kind="ExternalInput")
    BQ = nc.dram_tensor("bq_t", [L, 128, DC], F32, kind="ExternalInput")  # x0.125
    BK = nc.dram_tensor("bk_t", [L, 128, DC], F32, kind="ExternalInput")
    B1 = nc.dram_tensor("b1_t", [L, 128, FC], F32, kind="ExternalInput")
    BROW = nc.dram_tensor("brow", [L, 3, D], BF16, kind="ExternalInput")  # bv,bo,b2
    LNS = nc.dram_tensor("ln_s", [128, 2 * L + 1, DC], F32, kind="ExternalInput")
    LNB = nc.dram_tensor("ln_b", [128, 2 * L + 1, DC], F32, kind="ExternalInput")
    M01 = nc.dram_tensor("m01t", [128, KC, SQ], BF16, kind="ExternalInput")
    ONEH = nc.dram_tensor("oneh", [128, 4, HD], BF16, kind="ExternalInput")
    FCW = nc.dram_tensor("fcw", [128, DC, VSH], BF16, kind="ExternalInput")
    FCB = nc.dram_tensor("fcb", [1, VSH], BF16, kind="ExternalInput")
    OUT = nc.dram_tensor("out_t", [VSH // 128, 128, S], F32, kind="ExternalOutput")

    with tile.TileContext(nc) as tc:
        with tc.tile_pool(name="dram", bufs=1, space="DRAM") as dram, \
             tc.tile_pool(name="res", bufs=1) as res, \
             tc.tile_pool(name="const", bufs=1) as const:
            bnck_ins = [dram.tile([KT_FLAT], FP8, tag=f"cki{l}", name=f"cki{l}")
                        for l in range(L)]
            bnck_outs = [dram.tile([NC * KT_FLAT], FP8, addr_space="Shared",
                                   tag=f"cko{l}", name=f"cko{l}")
                         for l in range(L)]
            bncv_ins = [dram.tile([VN_FLAT], FP8, tag=f"cvi{l}", name=f"cvi{l}")
                        for l in range(L)]
            bncv_outs = [dram.tile([NC * VN_FLAT], FP8, addr_space="Shared",
                                   tag=f"cvo{l}", name=f"cvo{l}")
                         for l in range(L)]
            bncx_in = dram.tile([XF_FLAT], BF16)
            bncx_out = dram.tile([NC * XF_FLAT], BF16, addr_space="Shared")

            xT = res.tile([128, DC, SQ], F32)
            nc.sync.dma_start(xT[:], ET[:])
            m01 = const.tile([128, KC, SQ], BF16)
            nc.sync.dma_start(m01[:], M01[:])
            oneh = const.tile([128, 4, HD], BF16)
            nc.sync.dma_start(oneh[:], ONEH[:])
            lns = const.tile([128, 2 * L + 1, DC], F32)
            lnb = const.tile([128, 2 * L + 1, DC], F32)
            nc.sync.dma_start(lns[:], LNS[:])
            nc.sync.dma_start(lnb[:], LNB[:])
            ones_b = const.tile([128, 1], BF16)
            nc.vector.memset(ones_b[:], 1.0)
            ones_fr = const.tile([1, 128], F32)
            nc.vector.memset(ones_fr[:], 1.0)
            onesrow = const.tile([1, 512], BF16)
            nc.vector.memset(onesrow[:], 1.0)
            eps = const.tile([1, 1], F32)
            nc.vector.memset(eps[:], 1e-12)

            with tc.tile_pool(name="wp", bufs=1) as wp, \
                 tc.tile_pool(name="wf", bufs=2) as wf, \
                 tc.tile_pool(name="act", bufs=2) as act, \
                 tc.tile_pool(name="ag", bufs=1) as ag, \
                 tc.tile_pool(name="sm", bufs=2) as sm, \
                 tc.tile_pool(name="ps", bufs=2, space="PSUM") as ps, \
                 tc.tile_pool(name="pssc", bufs=2, space="PSUM") as pssc, \
                 tc.tile_pool(name="ps1", bufs=2, space="PSUM") as ps1:

                xbf = act.tile([128, DC, SQ], BF16, tag="xbf", bufs=1)
                # softmax denominator parking (head h -> partition 32*(h%4),
                # group h//4); memset once so full-tile reciprocal never sees
                # uninitialized bits
                dnt = sm.tile([128, 3, SQ], F32, tag="dnt", bufs=1)
                nc.vector.memset(dnt[:], 1.0)

                # ---- layernorm helpers (chunk-pipelined) ----
                def ln_stats_start():
                    su = pssc.tile([1, SQ], F32, tag="sc", name="lnsum")
                    sq = pssc.tile([1, SQ], F32, tag="sc", name="lnssq")
                    return su, sq

                def ln_stats_chunk(stats, oc, src_bf):
                    su, sq = stats
                    sqs = act.tile([128, SQ], BF16, tag="sqs")
                    nc.scalar.activation(sqs[:], src_bf, AF.Square)
                    nc.tensor.matmul(su[:], ones_b[:], src_bf,
                                     start=(oc == 0), stop=(oc == DC - 1))
                    nc.tensor.matmul(sq[:], ones_b[:], sqs[:],
                                     start=(oc == 0), stop=(oc == DC - 1))

                def ln_tail_apply(stats, li):
                    su, sq = stats
                    nmean = sm.tile([1, SQ], F32, tag="nmean")
                    ab = sm.tile([1, 2 * SQ], F32, tag="ab")
                    nc.vector.tensor_scalar_mul(nmean[:], su[:], -1.0 / D)
                    nc.vector.tensor_scalar_mul(ab[:, 0:SQ], sq[:], 1.0 / D)
                    t0 = sm.tile([1, SQ], F32, tag="lnt0")
                    nc.vector.tensor_tensor(t0[:], nmean[:], nmean[:], op=MUL)
                    nc.vector.tensor_tensor(ab[:, 0:SQ], ab[:, 0:SQ], t0[:], op=SUB)
                    # rstd = 1/sqrt(var + eps)
                    nc.scalar.activation(t0[:], ab[:, 0:SQ], AF.Sqrt, bias=eps[:])
                    nc.vector.reciprocal_approx_fast(ab[:, 0:SQ], t0[:])
                    nc.vector.tensor_tensor(ab[:, SQ:], nmean[:], ab[:, 0:SQ],
                                            op=MUL)              # -mean*rstd
                    bc = pssc.tile([128, 2 * SQ], F32, tag="sc", name="lnbc")
                    nc.tensor.matmul(bc[:], ones_fr[:], ab[:], start=True, stop=True)
                    # xbf (bf16, consumed by matmuls) is produced with the
                    # shortest possible vector chain; the fp32 xT update (+b)
                    # is deferred so it doesn't gate the next phase's PE work
                    t2s = []
                    for oc in range(DC):
                        t1 = sm.tile([128, SQ], F32, tag="lnt1")
                        nc.vector.scalar_tensor_tensor(
                            t1[:], xT[:, oc, :], lns[:, li, oc:oc + 1],
                            bc[:, 0:SQ], op0=MUL, op1=MUL)
                        t2 = sm.tile([128, SQ], F32, tag="lnt2", bufs=6)
                        nc.vector.scalar_tensor_tensor(
                            t2[:], bc[:, SQ:2 * SQ], lns[:, li, oc:oc + 1],
                            t1[:], op0=MUL, op1=ADD)
                        nc.scalar.activation(xbf[:, oc, :], t2[:], AF.Identity,
                                             bias=lnb[:, li, oc:oc + 1])
                        t2s.append(t2)
                    for oc in range(DC):
                        nc.vector.tensor_scalar_add(xT[:, oc, :], t2s[oc][:],
                                                    lnb[:, li, oc:oc + 1])

                # ---- embedding layernorm (standalone) ----
                wk = wp.tile([128, DC, D], BF16, tag="wk")
                nc.sync.dma_start(wk[:], WK[0])
                wv = wp.tile([128, DC, D], BF16, tag="wv")
                nc.sync.dma_start(wv[:], WV[0])
                wq = wp.tile([128, DC, D], BF16, tag="wq")
                nc.sync.dma_start(wq[:], WQ[0])
                bq = wp.tile([128, DC], F32, tag="bq")
                nc.sync.dma_start(bq[:], BQ[0])
                bk = wp.tile([128, DC], F32, tag="bk")
                nc.sync.dma_start(bk[:], BK[0])
                bvr = wp.tile([1, D], BF16, tag="bvr")
                nc.sync.dma_start(bvr[:], BROW[0][0:1, :])
                bor = wp.tile([1, D], BF16, tag="bor")
                nc.sync.dma_start(bor[:], BROW[0][1:2, :])
                b2r = wp.tile([1, D], BF16, tag="b2r")
                nc.sync.dma_start(b2r[:], BROW[0][2:3, :])
                wo = wp.tile([128, DC, D], BF16, tag="wo")
                nc.sync.dma_start(wo[:], WO[0])

                st0 = ln_stats_start()
                for oc in range(DC):
                    xs = act.tile([128, SQ], BF16, tag="xs")
                    nc.vector.tensor_copy(xs[:], xT[:, oc, :])
                    ln_stats_chunk(st0, oc, xs[:])
                ln_tail_apply(st0, 0)

                for l in range(L):
                    with nc.named_scope(f"L{l}"):
                        # ---- K^T projection + AG launch ----
                        ktb = act.tile([128, DC, SQ], FP8, tag="ktb", bufs=1)
                        for oc in range(DC):
                            kp = ps.tile([128, SQ], F32, tag="proj")
                            for dc in range(DC):
                                nc.tensor.matmul(
                                    kp[:], wk[:, dc, 128 * oc:128 * (oc + 1)],
                                    xbf[:, dc, :], start=(dc == 0),
                                    stop=(dc == DC - 1))
                            nc.scalar.activation(ktb[:, oc, :], kp[:],
                                                 AF.Identity,
                                                 bias=bk[:, oc:oc + 1])
                        nc.sync.dma_start(
                            bnck_ins[l][:].rearrange("(p i q) -> p i q",
                                                     p=128, i=DC), ktb[:])
                        nc.gpsimd.collective_compute(
                            "AllGather", mybir.AluOpType.bypass,
                            replica_groups=[list(range(NC))],
                            ins=[bnck_ins[l][:].opt()],
                            outs=[bnck_outs[l][:].opt()])

                        # ---- V normal + AG launch ----
                        vnb = act.tile([128, 2, VNF], FP8, tag="vnb", bufs=1)
                        nc.vector.memset(
                            vnb[:].rearrange("p rc (h e) -> p rc h e",
                                             h=H)[:, :, :, HD:], 1.0)
                        for rc in range(2):
                            for hf in range(2):
                                cs = slice(384 * hf, 384 * (hf + 1))
                                vp = ps.tile([128, 384], F32, tag="proj",
                                             name="vproj")
                                for dc in range(DC):
                                    nc.tensor.matmul(
                                        vp[:], xbf[:, dc, 128 * rc:128 * (rc + 1)],
                                        wv[:, dc, cs], start=(dc == 0), stop=False)
                                nc.tensor.matmul(vp[:], onesrow[:1, :128],
                                                 bvr[:1, cs], start=False, stop=True)
                                nc.vector.tensor_copy(
                                    vnb[:, rc, 390 * hf:390 * (hf + 1)].rearrange(
                                        "p (h e) -> p h e", h=6)[:, :, 0:HD],
                                    vp[:].rearrange("p (h e) -> p h e", h=6))
                        nc.sync.dma_start(
                            bncv_ins[l][:].rearrange("(p rc f) -> p rc f",
                                                     p=128, rc=2), vnb[:])
                        nc.gpsimd.collective_compute(
                            "AllGather", mybir.AluOpType.bypass,
                            replica_groups=[list(range(NC))],
                            ins=[bncv_ins[l][:].opt()],
                            outs=[bncv_outs[l][:].opt()])

                        # ---- Q^T (overlaps AG) ----
                        qtb = act.tile([128, DC, SQ], BF16, tag="qtb", bufs=1)
                        for oc in range(DC):
                            qp = ps.tile([128, SQ], F32, tag="proj")
                            for dc in range(DC):
                                nc.tensor.matmul(
                                    qp[:], wq[:, dc, 128 * oc:128 * (oc + 1)],
                                    xbf[:, dc, :], start=(dc == 0),
                                    stop=(dc == DC - 1))
                            nc.scalar.activation(qtb[:, oc, :], qp[:],
                                                 AF.Identity,
                                                 bias=bq[:, oc:oc + 1],
                                                 scale=0.125)

                        # prefetch next layer's QKV weights
                        if l + 1 < L:
                            wk = wp.tile([128, DC, D], BF16, tag="wk")
                            nc.sync.dma_start(wk[:], WK[l + 1])
                            wv = wp.tile([128, DC, D], BF16, tag="wv")
                            nc.sync.dma_start(wv[:], WV[l + 1])
                            wq = wp.tile([128, DC, D], BF16, tag="wq")
                            nc.sync.dma_start(wq[:], WQ[l + 1])
                            bq = wp.tile([128, DC], F32, tag="bq")
                            nc.sync.dma_start(bq[:], BQ[l + 1])
                            bk = wp.tile([128, DC], F32, tag="bk")
                            nc.sync.dma_start(bk[:], BK[l + 1])

                        # ---- AG copy-outs (2 halves each, gpsimd queue) ----
                        ktag = ag.tile([128, NC * DC, SQ], FP8, tag="ktag")
                        vnag = ag.tile([128, KC, VNF], FP8, tag="vnag")
                        for hh, eng in ((0, nc.sync), (1, nc.gpsimd)):
                            o = 4 * hh * KT_FLAT
                            eng.dma_start(
                                ktag[:, 24 * hh:24 * (hh + 1), :].rearrange(
                                    "p (r i) q -> p r i q", r=4),
                                bnck_outs[l][o:o + 4 * KT_FLAT].rearrange(
                                    "(r p i q) -> p r i q", r=4, p=128, i=DC))
                        for hh, eng in ((0, nc.sync), (1, nc.gpsimd)):
                            o = 4 * hh * VN_FLAT
                            eng.dma_start(
                                vnag[:, 8 * hh:8 * (hh + 1), :].rearrange(
                                    "p (r rc) f -> p r rc f", r=4),
                                bncv_outs[l][o:o + 4 * VN_FLAT].rearrange(
                                    "(r p rc f) -> p r rc f", r=4, p=128, rc=2))

                        # ---- attention: head pairs, interleaved row groups ----
                        ctxr = act.tile([128, DC, SQ], BF16, tag="ctxr", bufs=1)
                        ctxb = act.tile([128, DC, SQ], BF16, tag="ctxb", bufs=1)
                        for t in range(6):
                            hA, hB = 2 * t, 2 * t + 1
                            ctA = ps1.tile([VN_E, SQ], F32, tag="ctx")
                            ctB = ps1.tile([VN_E, SQ], F32, tag="ctx")
                            for w in range(4):
                                scA = pssc.tile([128, 4, SQ], F32, tag="sc")
                                scB = pssc.tile([128, 4, SQ], F32, tag="sc")
                                for j in range(4):
                                    kc = 4 * w + j
                                    r, hf = kc // 2, kc % 2
                                    nc.tensor.matmul(
                                        scA[:, j, :],
                                        ktag[0:64, DC * r + t,
                                             128 * hf:128 * (hf + 1)],
                                        qtb[0:64, t, :], start=True, stop=True)
                                    nc.tensor.matmul(
                                        scB[:, j, :],
                                        ktag[64:128, DC * r + t,
                                             128 * hf:128 * (hf + 1)],
                                        qtb[64:128, t, :], start=True, stop=True)
                                prA = sm.tile([128, 4, SQ], BF16, tag="pr",
                                              bufs=4)
                                prB = sm.tile([128, 4, SQ], BF16, tag="pr",
                                              bufs=4)
                                nc.scalar.activation(prA[:], scA[:], AF.Exp)
                                nc.scalar.activation(prB[:], scB[:], AF.Exp)
                                nc.vector.tensor_tensor(
                                    prA[:], prA[:], m01[:, 4 * w:4 * w + 4, :],
                                    op=MUL)
                                nc.vector.tensor_tensor(
                                    prB[:], prB[:], m01[:, 4 * w:4 * w + 4, :],
                                    op=MUL)
                                for j in range(4):
                                    kc = 4 * w + j
                                    nc.tensor.matmul(
                                        ctA[:], vnag[:, kc, 65 * hA:65 * hA + 65],
                                        prA[:, j, :], start=(kc == 0),
                                        stop=(kc == KC - 1))
                                    nc.tensor.matmul(
                                        ctB[:], vnag[:, kc, 65 * hB:65 * hB + 65],
                                        prB[:, j, :], start=(kc == 0),
                                        stop=(kc == KC - 1))
                            pA, gA = 32 * (hA % 4), hA // 4
                            pB, gB = 32 * (hB % 4), hB // 4
                            nc.vector.tensor_copy(dnt[pA:pA + 1, gA, :],
                                                  ctA[HD:HD + 1, :])
                            nc.vector.tensor_copy(dnt[pB:pB + 1, gB, :],
                                                  ctB[HD:HD + 1, :])
                            nc.vector.tensor_copy(ctxr[0:64, t, :], ctA[0:HD, :])
                            nc.vector.tensor_copy(ctxr[64:128, t, :], ctB[0:HD, :])
                        rcpf = sm.tile([128, 3, SQ], F32, tag="rcpf", bufs=1)
                        nc.vector.reciprocal_approx_fast(rcpf[:], dnt[:])
                        rcp = sm.tile([128, 3, SQ], BF16, tag="rcp", bufs=1)
                        nc.vector.tensor_copy(rcp[:], rcpf[:])
                        for h in range(H):
                            ki, kp_ = h // 2, 64 * (h % 2)
                            rbc = ps.tile([64, SQ], F32, tag="proj", name="rbc")
                            nc.tensor.matmul(rbc[:], oneh[:, h % 4, :],
                                             rcp[:, h // 4, :],
                                             start=True, stop=True)
                            nc.vector.tensor_tensor(ctxb[kp_:kp_ + 64, ki, :],
                                                    ctxr[kp_:kp_ + 64, ki, :],
                                                    rbc[:], op=MUL)

                        # ---- Wo + residual + LN1 stats (chunked) ----
                        st1 = ln_stats_start()
                        for oc in range(DC):
                            op_ = ps.tile([128, SQ], F32, tag="proj")
                            for dc in range(DC):
                                nc.tensor.matmul(
                                    op_[:], wo[:, dc, 128 * oc:128 * (oc + 1)],
                                    ctxb[:, dc, :], start=(dc == 0), stop=False)
                            nc.tensor.matmul(op_[:],
                                             bor[:1, 128 * oc:128 * (oc + 1)],
                                             onesrow[:1, :SQ], start=False,
                                             stop=True)
                            xs = act.tile([128, SQ], BF16, tag="xs")
                            nc.vector.scalar_tensor_tensor(
                                xs[:], xT[:, oc, :], 1.0, op_[:],
                                op0=MUL, op1=ADD)
                            nc.vector.tensor_tensor(xT[:, oc, :], xT[:, oc, :],
                                                    op_[:], op=ADD)
                            ln_stats_chunk(st1, oc, xs[:])
                        ln_tail_apply(st1, 1 + 2 * l)

                        # ---- FFN ----
                        b1 = wp.tile([128, FC], F32, tag="b1")
                        nc.sync.dma_start(b1[:], B1[l])
                        htb = act.tile([128, FC, SQ], BF16, tag="htb", bufs=1)
                        for qi in range(6):
                            w1q = wf.tile([128, DC, F // 6], BF16, tag="w1q")
                            nc.sync.dma_start(
                                w1q[:], W1[l][:, :, (F // 6) * qi:(F // 6) * (qi + 1)])
                            for oc6 in range(4):
                                oc = 4 * qi + oc6
                                hp = ps.tile([128, SQ], F32, tag="proj")
                                for dc in range(DC):
                                    nc.tensor.matmul(
                                        hp[:], w1q[:, dc, 128 * oc6:128 * (oc6 + 1)],
                                        xbf[:, dc, :], start=(dc == 0),
                                        stop=(dc == DC - 1))
                                nc.scalar.activation(htb[:, oc, :], hp[:],
                                                     AF.Gelu_apprx_tanh,
                                                     bias=b1[:, oc:oc + 1])
                        st2 = ln_stats_start()
                        for hi in range(3):
                            w2h = wf.tile([128, FC, D // 3], BF16, tag="w2h")
                            nc.sync.dma_start(
                                w2h[:], W2[l][:, :, (D // 3) * hi:(D // 3) * (hi + 1)])
                            for oc3 in range(2):
                                oc = 2 * hi + oc3
                                yp = ps.tile([128, SQ], F32, tag="proj")
                                for kc in range(FC):
                                    nc.tensor.matmul(
                                        yp[:], w2h[:, kc, 128 * oc3:128 * (oc3 + 1)],
                                        htb[:, kc, :], start=(kc == 0), stop=False)
                                nc.tensor.matmul(
                                    yp[:], b2r[:1, 128 * oc:128 * (oc + 1)],
                                    onesrow[:1, :SQ], start=False, stop=True)
                                xs = act.tile([128, SQ], BF16, tag="xs")
                                nc.vector.scalar_tensor_tensor(
                                    xs[:], xT[:, oc, :], 1.0, yp[:],
                                    op0=MUL, op1=ADD)
                                nc.vector.tensor_tensor(xT[:, oc, :], xT[:, oc, :],
                                                        yp[:], op=ADD)
                                ln_stats_chunk(st2, oc, xs[:])
                        ln_tail_apply(st2, 2 + 2 * l)
                        if l + 1 < L:
                            bvr = wp.tile([1, D], BF16, tag="bvr")
                            nc.sync.dma_start(bvr[:], BROW[l + 1][0:1, :])
                            bor = wp.tile([1, D], BF16, tag="bor")
                            nc.sync.dma_start(bor[:], BROW[l + 1][1:2, :])
                            b2r = wp.tile([1, D], BF16, tag="b2r")
                            nc.sync.dma_start(b2r[:], BROW[l + 1][2:3, :])
                            wo = wp.tile([128, DC, D], BF16, tag="wo")
                            nc.sync.dma_start(wo[:], WO[l + 1])

            with tc.tile_pool(name="fin", bufs=1) as fin, \
                 tc.tile_pool(name="fw", bufs=4) as fw, \
                 tc.tile_pool(name="fo", bufs=3) as fo, \
                 tc.tile_pool(name="fps", bufs=2, space="PSUM") as fps:
                xbf2 = fin.tile([128, DC, SQ], BF16)
                nc.vector.tensor_copy(xbf2[:], xT[:])
                nc.sync.dma_start(
                    bncx_in[:].rearrange("(p i q) -> p i q", p=128, i=DC), xbf2[:])
                nc.gpsimd.collective_compute(
                    "AllGather", mybir.AluOpType.bypass,
                    replica_groups=[list(range(NC))],
                    ins=[bncx_in[:].opt()], outs=[bncx_out[:].opt()])
                xf = fin.tile([128, DC, S], BF16)
                nc.gpsimd.dma_start(
                    xf[:].rearrange("p i (r q) -> p i r q", r=NC),
                    bncx_out[:].rearrange("(r p i q) -> p i r q",
                                          r=NC, p=128, i=DC))
                fcb = fin.tile([1, VSH], BF16)
                nc.sync.dma_start(fcb[:], FCB[:])
                for oc in range(VSH // 128):
                    fwt = fw.tile([128, DC, 128], BF16, tag="fwt")
                    nc.sync.dma_start(fwt[:], FCW[:, :, 128 * oc:128 * (oc + 1)])
                    fp = fps.tile([128, S], F32, tag="fp")
                    for ncol in range(4):
                        cs = slice(512 * ncol, 512 * (ncol + 1))
                        for dc in range(DC):
                            nc.tensor.matmul(fp[:, cs], fwt[:, dc, :],
                                             xf[:, dc, cs], start=(dc == 0),
                                             stop=False)
                        nc.tensor.matmul(fp[:, cs],
                                         fcb[0:1, 128 * oc:128 * (oc + 1)],
                                         onesrow[:1, :], start=False, stop=True)
                    ot = fo.tile([128, S], F32, tag="ot")
                    if oc % 2 == 0:
                        nc.scalar.activation(ot[:], fp[:], AF.Copy)
                    else:
                        nc.vector.tensor_copy(ot[:], fp[:])
                    nc.sync.dma_start(OUT[oc], ot[:])
    nc.finalize()
    return nc


def kernel(input_ids, attention_mask, token_type_ids, word_emb, pos_emb, type_emb,
           emb_ln_s, emb_ln_b, Wq, Wk, Wv, bq, bk, bv, Wo, bo, ln1_s, ln1_b,
           W1, b1, W2, b2, ln2_s, ln2_b, fc_w, fc_b):
    f32 = np.float32
    bf = ml_dtypes.bfloat16
    ids = np.asarray(input_ids)[0]
    tt = np.asarray(token_type_ids)[0]
    am = np.asarray(attention_mask)[0].astype(f32)
    E = (np.asarray(word_emb, f32)[ids] + np.asarray(pos_emb, f32)
         + np.asarray(type_emb, f32)[tt])

    def tp(w):  # [D, N] -> [128, DC, N] bf16
        return np.ascontiguousarray(
            np.asarray(w, f32).reshape(-1, 128, w.shape[-1]).transpose(1, 0, 2)
        ).astype(bf)

    def col(b, scale=1.0):  # [N*128] -> [128, N] f32
        b = np.asarray(b, f32) * scale
        return np.ascontiguousarray(b.reshape(-1, 128).T).astype(f32)

    wq_h = np.stack([tp(Wq[l]) for l in range(L)])
    wk_h = np.stack([tp(Wk[l]) for l in range(L)])
    wv_h = np.stack([tp(Wv[l]) for l in range(L)])
    wo_h = np.stack([tp(Wo[l]) for l in range(L)])
    w1_h = np.stack([tp(W1[l]) for l in range(L)])
    w2_h = np.stack([tp(W2[l]) for l in range(L)])
    bq_h = np.stack([col(bq[l], 0.125) for l in range(L)])
    bk_h = np.stack([col(bk[l]) for l in range(L)])
    b1_h = np.stack([col(b1[l]) for l in range(L)])
    brow_h = np.stack([np.stack([np.asarray(bv[l], f32), np.asarray(bo[l], f32),
                                 np.asarray(b2[l], f32)]) for l in range(L)]).astype(bf)

    lns_list = [np.asarray(emb_ln_s, f32)]
    lnb_list = [np.asarray(emb_ln_b, f32)]
    for l in range(L):
        lns_list += [np.asarray(ln1_s[l], f32), np.asarray(ln2_s[l], f32)]
        lnb_list += [np.asarray(ln1_b[l], f32), np.asarray(ln2_b[l], f32)]
    ln_s = np.stack([s.reshape(DC, 128).T for s in lns_list], axis=1)
    ln_b = np.stack([s.reshape(DC, 128).T for s in lnb_list], axis=1)

    blk = _block_map(NB, R)
    allowed = np.zeros((NB, NB), f32)
    for i in range(NB):
        allowed[i, blk[i]] = 1.0
    allowed[0, :] = 1.0
    allowed[NB - 1, :] = 1.0
    allow_key = np.repeat(allowed, BS, axis=1) * am[None, :]

    oneh_h = np.zeros((128, 4, HD), f32)
    for k in range(4):
        oneh_h[32 * k, k, :] = 1.0
    oneh_h = oneh_h.astype(bf)

    fcw_pad = np.zeros((D, VSH * NC), f32)
    fcw_pad[:, :V] = np.asarray(fc_w, f32)
    fcb_pad = np.zeros((VSH * NC,), f32)
    fcb_pad[:V] = np.asarray(fc_b, f32)

    if "k" not in _nc_cache:
        _nc_cache["k"] = build()
    nc = _nc_cache["k"]

    in_maps = []
    for c in range(NC):
        e_shard = E[SQ * c:SQ * (c + 1)].T.reshape(DC, 128, SQ).transpose(1, 0, 2)
        # flat mask [128, KC, SQ]: m01[p, kc, q] = allow(qblock(q), key kc*128+p)
        m01 = np.repeat(
            allow_key[4 * c:4 * c + 4].T.reshape(KC, 128, 4).transpose(1, 0, 2),
            BS, axis=2)
        in_maps.append({
            "e_t": np.ascontiguousarray(e_shard).astype(f32),
            "wq": wq_h, "wk": wk_h, "wv": wv_h, "wo": wo_h,
            "w1": w1_h, "w2": w2_h,
            "bq_t": bq_h, "bk_t": bk_h, "b1_t": b1_h, "brow": brow_h,
            "ln_s": ln_s, "ln_b": ln_b,
            "m01t": np.ascontiguousarray(m01).astype(bf),
            "oneh": oneh_h,
            "fcw": tp(fcw_pad[:, VSH * c:VSH * (c + 1)]),
            "fcb": fcb_pad[None, VSH * c:VSH * (c + 1)].astype(bf),
        })

    trace = bool(int(os.environ.get("BB_TRACE", "0")))
    res = run_bass_kernel_spmd(nc, in_maps, core_ids=list(range(NC)), trace=trace)
    kernel.last_exec_ns = getattr(res, "exec_time_ns", None)
    kernel.last_result = res
    outs = [res.results[c]["out_t"].reshape(VSH, S).T for c in range(NC)]
    logits = np.concatenate(outs, axis=1)[:, :V]
    return logits[None].astype(np.float32)


# revision 25
# speedup vs baseline: 1.0708x; 1.0503x over previous
"""BigBird encoder + vocab projection on 8 Trainium2 NeuronCores.

Sequence-sharded transformer (core c owns rows [256c, 256c+256) = 4 query
blocks), per-layer split AllGather of K^T then V (overlapped with Q proj +
attention), vocab-sharded final projection.  Activations live transposed
(xT [768part, 256free]) so every matmul consumes weights as stored.
BigBird sparsity (window + global + random + dedup + key_mask) is folded
into a per-core 0/1 mask multiplied into exp(scores) — mathematically
identical to the reference's gather+softmax.  bf16 matmuls, fp32
accumulate/residual.

Perf structure (v2): K-AG and V-AG launched as soon as each projection
finishes; copy-outs are two large p-major DMAs each; score matmuls of head
pairs interleave on PE row-groups 0-63/64-127 for 2x concurrency; softmax
reciprocals batched into one [12,SQ] op; LayerNorm stats accumulate
chunk-wise inside the residual loops to keep PE busy (HAM warm).
"""
import os, sys
os.environ.setdefault("JAX_PLATFORMS", "")
import numpy as np
import ml_dtypes

sys.path.insert(0, "/opt/trn_rl_repo")

import concourse.bass as bass
import concourse.tile as tile
from concourse import bacc, mybir
from concourse.bass_utils import run_bass_kernel_spmd

BF16 = mybir.dt.bfloat16
FP8 = mybir.dt.float8e4
F32 = mybir.dt.float32
AF = mybir.ActivationFunctionType
MUL = mybir.AluOpType.mult
ADD = mybir.AluOpType.add
SUB = mybir.AluOpType.subtract

B, S, D, F, V = 1, 2048, 768, 3072, 50358
L, H, HD, BS, NB, R = 12, 12, 64, 64, 32, 3
NC = 8
SQ = S // NC                # 256
DC = D // 128               # 6
FC = F // 128               # 24
KC = S // 128               # 16
VSH = 6400                  # padded vocab shard (50 x 128)
VN_E = HD + 1               # 65 cols per head in V-normal (ones col for rowsum)
VNF = H * VN_E              # 780 flat per (p, rc)
VN_FLAT = 128 * 2 * VNF     # 199680
KT_FLAT = 128 * DC * SQ     # 196608
XF_FLAT = 128 * DC * SQ

_nc_cache = {}


def _block_map(nb, r, seed=0):
    rng = np.random.default_rng(seed)
    idx = np.zeros((nb, 5 + r), np.int32)
    for i in range(nb):
        lst = [0, nb - 1, max(i - 1, 0), i, min(i + 1, nb - 1)]
        cand = np.setdiff1d(np.arange(nb), np.array(lst))
        lst += list(rng.choice(cand, r, replace=False))
        for j, b in enumerate(lst):
            idx[i, j] = int(b)
    return idx


def build():
    nc = bacc.Bacc("TRN2", target_bir_lowering=False, debug=False, num_devices=NC)
    ET = nc.dram_tensor("e_t", [128, DC, SQ], F32, kind="ExternalInput")
    WQ = nc.dram_tensor("wq", [L, 128, DC, D], BF16, kind="ExternalInput")
    WK = nc.dram_tensor("wk", [L, 128, DC, D], BF16, kind="ExternalInput")
    WV = nc.dram_tensor("wv", [L, 128, DC, D], BF16, kind="ExternalInput")
    WO = nc.dram_tensor("wo", [L, 128, DC, D], BF16, kind="ExternalInput")
    W1 = nc.dram_tensor("w1", [L, 128, DC, F], BF16, kind="ExternalInput")
    W2 = nc.dram_tensor("w2", [L, 128, FC, D], BF16, kind="ExternalInput")
    BQ = nc.dram_tensor("bq_t", [L, 128, DC], F32, kind="ExternalInput")  # x0.125
    BK = nc.dram_tensor("bk_t", [L, 128, DC], F32, kind="ExternalInput")
    B1 = nc.dram_tensor("b1_t", [L, 128, FC], F32, kind="ExternalInput")
    BROW = nc.dram_tensor("brow", [L, 3, D], BF16, kind="ExternalInput")  # bv,bo,b2
    LNS = nc.dram_tensor("ln_s", [128, 2 * L + 1, DC], F32, kind="ExternalInput")
    LNB = nc.dram_tensor("ln_b", [128, 2 * L + 1, DC], F32, kind="ExternalInput")
    M01 = nc.dram_tensor("m01t", [128, KC, SQ], BF16, kind="ExternalInput")
    ONEH = nc.dram_tensor("oneh", [128, 4, HD], BF16, kind="ExternalInput")
    FCW = nc.dram_tensor("fcw", [128, DC, VSH], BF16, kind="ExternalInput")
    FCB = nc.dram_tensor("fcb", [1, VSH], BF16, kind="ExternalInput")
    OUT = nc.dram_tensor("out_t", [VSH // 128, 128, S], F32, kind="ExternalOutput")

    with tile.TileContext(nc) as tc:
        with tc.tile_pool(name="dram", bufs=1, space="DRAM") as dram, \
             tc.tile_pool(name="res", bufs=1) as res, \
             tc.tile_pool(name="const", bufs=1) as const:
            AGF = KT_FLAT + VN_FLAT
            bnc_ins = [dram.tile([AGF], FP8, tag=f"cki{l}", name=f"cki{l}")
                       for l in range(L)]
            bnc_outs = [dram.tile([NC * AGF], FP8, addr_space="Shared",
                                  tag=f"cko{l}", name=f"cko{l}")
                        for l in range(L)]
            bncx_in = dram.tile([XF_FLAT], BF16)
            bncx_out = dram.tile([NC * XF_FLAT], BF16, addr_space="Shared")

            xT = res.tile([128, DC, SQ], F32)
            nc.sync.dma_start(xT[:], ET[:])
            m01 = const.tile([128, KC, SQ], BF16)
            nc.sync.dma_start(m01[:], M01[:])
            oneh = const.tile([128, 4, HD], BF16)
            nc.sync.dma_start(oneh[:], ONEH[:])
            lns = const.tile([128, 2 * L + 1, DC], F32)
            lnb = const.tile([128, 2 * L + 1, DC], F32)
            nc.sync.dma_start(lns[:], LNS[:])
            nc.sync.dma_start(lnb[:], LNB[:])
            ones_b = const.tile([128, 1], BF16)
            nc.vector.memset(ones_b[:], 1.0)
            ones_fr = const.tile([1, 128], F32)
            nc.vector.memset(ones_fr[:], 1.0)
            onesrow = const.tile([1, 512], BF16)
            nc.vector.memset(onesrow[:], 1.0)
            eps = const.tile([1, 1], F32)
            nc.vector.memset(eps[:], 1e-12)

            with tc.tile_pool(name="wp", bufs=1) as wp, \
                 tc.tile_pool(name="wf", bufs=2) as wf, \
                 tc.tile_pool(name="act", bufs=2) as act, \
                 tc.tile_pool(name="ag", bufs=1) as ag, \
                 tc.tile_pool(name="sm", bufs=2) as sm, \
                 tc.tile_pool(name="ps", bufs=2, space="PSUM") as ps, \
                 tc.tile_pool(name="pssc", bufs=2, space="PSUM") as pssc, \
                 tc.tile_pool(name="ps1", bufs=2, space="PSUM") as ps1:

                xbf = act.tile([128, DC, SQ], BF16, tag="xbf", bufs=1)
                # softmax denominator parking (head h -> partition 32*(h%4),
                # group h//4); memset once so full-tile reciprocal never sees
                # uninitialized bits
                dnt = sm.tile([128, 3, SQ], F32, tag="dnt", bufs=1)
                nc.vector.memset(dnt[:], 1.0)

                # ---- layernorm helpers (chunk-pipelined) ----
                def ln_stats_start():
                    su = pssc.tile([1, SQ], F32, tag="sc", name="lnsum")
                    sq = pssc.tile([1, SQ], F32, tag="sc", name="lnssq")
                    return su, sq

                def ln_stats_chunk(stats, oc, src_bf):
                    su, sq = stats
                    sqs = act.tile([128, SQ], BF16, tag="sqs")
                    nc.scalar.activation(sqs[:], src_bf, AF.Square)
                    nc.tensor.matmul(su[:], ones_b[:], src_bf,
                                     start=(oc == 0), stop=(oc == DC - 1))
                    nc.tensor.matmul(sq[:], ones_b[:], sqs[:],
                                     start=(oc == 0), stop=(oc == DC - 1))

                def ln_tail_apply(stats, li):
                    su, sq = stats
                    nmean = sm.tile([1, SQ], F32, tag="nmean")
                    ab = sm.tile([1, 2 * SQ], F32, tag="ab")
                    nc.vector.tensor_scalar_mul(nmean[:], su[:], -1.0 / D)
                    nc.vector.tensor_scalar_mul(ab[:, 0:SQ], sq[:], 1.0 / D)
                    t0 = sm.tile([1, SQ], F32, tag="lnt0")
                    nc.vector.tensor_tensor(t0[:], nmean[:], nmean[:], op=MUL)
                    nc.vector.tensor_tensor(ab[:, 0:SQ], ab[:, 0:SQ], t0[:], op=SUB)
                    # rstd = 1/sqrt(var + eps)
                    nc.scalar.activation(t0[:], ab[:, 0:SQ], AF.Sqrt, bias=eps[:])
                    nc.vector.reciprocal_approx_fast(ab[:, 0:SQ], t0[:])
                    nc.vector.tensor_tensor(ab[:, SQ:], nmean[:], ab[:, 0:SQ],
                                            op=MUL)              # -mean*rstd
                    bc = pssc.tile([128, 2 * SQ], F32, tag="sc", name="lnbc")
                    nc.tensor.matmul(bc[:], ones_fr[:], ab[:], start=True, stop=True)
                    # xbf (bf16, consumed by matmuls) is produced with the
                    # shortest possible vector chain; the fp32 xT update (+b)
                    # is deferred so it doesn't gate the next phase's PE work
                    t2s = []
                    for oc in range(DC):
                        t1 = sm.tile([128, SQ], F32, tag="lnt1")
                        nc.vector.scalar_tensor_tensor(
                            t1[:], xT[:, oc, :], lns[:, li, oc:oc + 1],
                            bc[:, 0:SQ], op0=MUL, op1=MUL)
                        t2 = sm.tile([128, SQ], F32, tag="lnt2", bufs=6)
                        nc.vector.scalar_tensor_tensor(
                            t2[:], bc[:, SQ:2 * SQ], lns[:, li, oc:oc + 1],
                            t1[:], op0=MUL, op1=ADD)
                        nc.scalar.activation(xbf[:, oc, :], t2[:], AF.Identity,
                                             bias=lnb[:, li, oc:oc + 1])
                        t2s.append(t2)
                    for oc in range(DC):
                        nc.vector.tensor_scalar_add(xT[:, oc, :], t2s[oc][:],
                                                    lnb[:, li, oc:oc + 1])

                # ---- embedding layernorm (standalone) ----
                wk = wp.tile([128, DC, D], BF16, tag="wk")
                nc.sync.dma_start(wk[:], WK[0])
                wv = wp.tile([128, DC, D], BF16, tag="wv")
                nc.sync.dma_start(wv[:], WV[0])
                wq = wp.tile([128, DC, D], BF16, tag="wq")
                nc.sync.dma_start(wq[:], WQ[0])
                bq = wp.tile([128, DC], F32, tag="bq")
                nc.sync.dma_start(bq[:], BQ[0])
                bk = wp.tile([128, DC], F32, tag="bk")
                nc.sync.dma_start(bk[:], BK[0])
                bvr = wp.tile([1, D], BF16, tag="bvr")
                nc.sync.dma_start(bvr[:], BROW[0][0:1, :])
                bor = wp.tile([1, D], BF16, tag="bor")
                nc.sync.dma_start(bor[:], BROW[0][1:2, :])
                b2r = wp.tile([1, D], BF16, tag="b2r")
                nc.sync.dma_start(b2r[:], BROW[0][2:3, :])
                wo = wp.tile([128, DC, D], BF16, tag="wo")
                nc.sync.dma_start(wo[:], WO[0])

                st0 = ln_stats_start()
                for oc in range(DC):
                    xs = act.tile([128, SQ], BF16, tag="xs")
                    nc.vector.tensor_copy(xs[:], xT[:, oc, :])
                    ln_stats_chunk(st0, oc, xs[:])
                ln_tail_apply(st0, 0)

                for l in range(L):
                    with nc.named_scope(f"L{l}"):
                        # ---- K^T projection + AG launch ----
                        ktb = act.tile([128, DC, SQ], FP8, tag="ktb", bufs=1)
                        for oc in range(DC):
                            kp = ps.tile([128, SQ], F32, tag="proj")
                            for dc in range(DC):
                                nc.tensor.matmul(
                                    kp[:], wk[:, dc, 128 * oc:128 * (oc + 1)],
                                    xbf[:, dc, :], start=(dc == 0),
                                    stop=(dc == DC - 1))
                            nc.scalar.activation(ktb[:, oc, :], kp[:],
                                                 AF.Identity,
                                                 bias=bk[:, oc:oc + 1])
                        nc.sync.dma_start(
                            bnc_ins[l][0:KT_FLAT].rearrange("(p i q) -> p i q",
                                                            p=128, i=DC), ktb[:])

                        # ---- V normal + AG launch ----
                        vnb = act.tile([128, 2, VNF], FP8, tag="vnb", bufs=1)
                        nc.vector.memset(
                            vnb[:].rearrange("p rc (h e) -> p rc h e",
                                             h=H)[:, :, :, HD:], 1.0)
                        for rc in range(2):
                            for hf in range(2):
                                cs = slice(384 * hf, 384 * (hf + 1))
                                vp = ps.tile([128, 384], F32, tag="proj",
                                             name="vproj")
                                for dc in range(DC):
                                    nc.tensor.matmul(
                                        vp[:], xbf[:, dc, 128 * rc:128 * (rc + 1)],
                                        wv[:, dc, cs], start=(dc == 0), stop=False)
                                nc.tensor.matmul(vp[:], onesrow[:1, :128],
                                                 bvr[:1, cs], start=False, stop=True)
                                nc.vector.tensor_copy(
                                    vnb[:, rc, 390 * hf:390 * (hf + 1)].rearrange(
                                        "p (h e) -> p h e", h=6)[:, :, 0:HD],
                                    vp[:].rearrange("p (h e) -> p h e", h=6))
                        nc.sync.dma_start(
                            bnc_ins[l][KT_FLAT:].rearrange("(p rc f) -> p rc f",
                                                           p=128, rc=2), vnb[:])
                        nc.gpsimd.collective_compute(
                            "AllGather", mybir.AluOpType.bypass,
                            replica_groups=[list(range(NC))],
                            ins=[bnc_ins[l][:].opt()],
                            outs=[bnc_outs[l][:].opt()])

                        # ---- Q^T (overlaps AG) ----
                        qtb = act.tile([128, DC, SQ], BF16, tag="qtb", bufs=1)
                        for oc in range(DC):
                            qp = ps.tile([128, SQ], F32, tag="proj")
                            for dc in range(DC):
                                nc.tensor.matmul(
                                    qp[:], wq[:, dc, 128 * oc:128 * (oc + 1)],
                                    xbf[:, dc, :], start=(dc == 0),
                                    stop=(dc == DC - 1))
                            nc.scalar.activation(qtb[:, oc, :], qp[:],
                                                 AF.Identity,
                                                 bias=bq[:, oc:oc + 1],
                                                 scale=0.125)

                        # prefetch next layer's QKV weights
                        if l + 1 < L:
                            wk = wp.tile([128, DC, D], BF16, tag="wk")
                            nc.sync.dma_start(wk[:], WK[l + 1])
                            wv = wp.tile([128, DC, D], BF16, tag="wv")
                            nc.sync.dma_start(wv[:], WV[l + 1])
                            wq = wp.tile([128, DC, D], BF16, tag="wq")
                            nc.sync.dma_start(wq[:], WQ[l + 1])
                            bq = wp.tile([128, DC], F32, tag="bq")
                            nc.sync.dma_start(bq[:], BQ[l + 1])
                            bk = wp.tile([128, DC], F32, tag="bk")
                            nc.sync.dma_start(bk[:], BK[l + 1])

                        # ---- AG copy-outs (2 halves each, gpsimd queue) ----
                        ktag = ag.tile([128, NC * DC, SQ], FP8, tag="ktag")
                        vnag = ag.tile([128, KC, VNF], FP8, tag="vnag")
                        bout = bnc_outs[l][:].rearrange("(r f) -> r f", r=NC)
                        for hh, eng in ((0, nc.sync), (1, nc.gpsimd)):
                            eng.dma_start(
                                ktag[:, 24 * hh:24 * (hh + 1), :].rearrange(
                                    "p (r i) q -> p r i q", r=4),
                                bout[4 * hh:4 * (hh + 1), 0:KT_FLAT].rearrange(
                                    "r (p i q) -> p r i q", p=128, i=DC))
                        for hh, eng in ((0, nc.sync), (1, nc.gpsimd)):
                            eng.dma_start(
                                vnag[:, 8 * hh:8 * (hh + 1), :].rearrange(
                                    "p (r rc) f -> p r rc f", r=4),
                                bout[4 * hh:4 * (hh + 1), KT_FLAT:].rearrange(
                                    "r (p rc f) -> p r rc f", p=128, rc=2))

                        # ---- attention: head pairs, interleaved row groups ----
                        ctxr = act.tile([128, DC, SQ], BF16, tag="ctxr", bufs=1)
                        ctxb = act.tile([128, DC, SQ], BF16, tag="ctxb", bufs=1)
                        for t in range(6):
                            hA, hB = 2 * t, 2 * t + 1
                            ctA = ps1.tile([VN_E, SQ], F32, tag="ctx")
                            ctB = ps1.tile([VN_E, SQ], F32, tag="ctx")
                            for w in range(4):
                                scA = pssc.tile([128, 4, SQ], F32, tag="sc")
                                scB = pssc.tile([128, 4, SQ], F32, tag="sc")
                                for j in range(4):
                                    kc = 4 * w + j
                                    r, hf = kc // 2, kc % 2
                                    nc.tensor.matmul(
                                        scA[:, j, :],
                                        ktag[0:64, DC * r + t,
                                             128 * hf:128 * (hf + 1)],
                                        qtb[0:64, t, :], start=True, stop=True)
                                    nc.tensor.matmul(
                                        scB[:, j, :],
                                        ktag[64:128, DC * r + t,
                                             128 * hf:128 * (hf + 1)],
                                        qtb[64:128, t, :], start=True, stop=True)
                                prA = sm.tile([128, 4, SQ], BF16, tag="pr",
                                              bufs=8)
                                prB = sm.tile([128, 4, SQ], BF16, tag="pr",
                                              bufs=8)
                                nc.scalar.activation(prA[:], scA[:], AF.Exp)
                                nc.scalar.activation(prB[:], scB[:], AF.Exp)
                                nc.vector.tensor_tensor(
                                    prA[:], prA[:], m01[:, 4 * w:4 * w + 4, :],
                                    op=MUL)
                                nc.vector.tensor_tensor(
                                    prB[:], prB[:], m01[:, 4 * w:4 * w + 4, :],
                                    op=MUL)
                                for j in range(4):
                                    kc = 4 * w + j
                                    nc.tensor.matmul(
                                        ctA[:], vnag[:, kc, 65 * hA:65 * hA + 65],
                                        prA[:, j, :], start=(kc == 0),
                                        stop=(kc == KC - 1))
                                    nc.tensor.matmul(
                                        ctB[:], vnag[:, kc, 65 * hB:65 * hB + 65],
                                        prB[:, j, :], start=(kc == 0),
                                        stop=(kc == KC - 1))
                            pA, gA = 32 * (hA % 4), hA // 4
                            pB, gB = 32 * (hB % 4), hB // 4
                            nc.vector.tensor_copy(dnt[pA:pA + 1, gA, :],
                                                  ctA[HD:HD + 1, :])
                            nc.vector.tensor_copy(dnt[pB:pB + 1, gB, :],
                                                  ctB[HD:HD + 1, :])
                            nc.vector.tensor_copy(ctxr[0:64, t, :], ctA[0:HD, :])
                            nc.vector.tensor_copy(ctxr[64:128, t, :], ctB[0:HD, :])
                        rcpf = sm.tile([128, 3, SQ], F32, tag="rcpf", bufs=1)
                        nc.vector.reciprocal_approx_fast(rcpf[:], dnt[:])
                        rcp = sm.tile([128, 3, SQ], BF16, tag="rcp", bufs=1)
                        nc.vector.tensor_copy(rcp[:], rcpf[:])
                        for h in range(H):
                            ki, kp_ = h // 2, 64 * (h % 2)
                            rbc = ps.tile([64, SQ], F32, tag="proj", name="rbc")
                            nc.tensor.matmul(rbc[:], oneh[:, h % 4, :],
                                             rcp[:, h // 4, :],
                                             start=True, stop=True)
                            nc.vector.tensor_tensor(ctxb[kp_:kp_ + 64, ki, :],
                                                    ctxr[kp_:kp_ + 64, ki, :],
                                                    rbc[:], op=MUL)

                        # ---- Wo + residual + LN1 stats (chunked) ----
                        st1 = ln_stats_start()
                        for oc in range(DC):
                            op_ = ps.tile([128, SQ], F32, tag="proj")
                            for dc in range(DC):
                                nc.tensor.matmul(
                                    op_[:], wo[:, dc, 128 * oc:128 * (oc + 1)],
                                    ctxb[:, dc, :], start=(dc == 0), stop=False)
                            nc.tensor.matmul(op_[:],
                                             bor[:1, 128 * oc:128 * (oc + 1)],
                                             onesrow[:1, :SQ], start=False,
                                             stop=True)
                            xs = act.tile([128, SQ], BF16, tag="xs")
                            nc.vector.scalar_tensor_tensor(
                                xs[:], xT[:, oc, :], 1.0, op_[:],
                                op0=MUL, op1=ADD)
                            nc.vector.tensor_tensor(xT[:, oc, :], xT[:, oc, :],
                                                    op_[:], op=ADD)
                            ln_stats_chunk(st1, oc, xs[:])
                        ln_tail_apply(st1, 1 + 2 * l)

                        # ---- FFN ----
                        b1 = wp.tile([128, FC], F32, tag="b1")
                        nc.sync.dma_start(b1[:], B1[l])
                        htb = act.tile([128, FC, SQ], BF16, tag="htb", bufs=1)
                        for qi in range(6):
                            w1q = wf.tile([128, DC, F // 6], BF16, tag="w1q")
                            nc.sync.dma_start(
                                w1q[:], W1[l][:, :, (F // 6) * qi:(F // 6) * (qi + 1)])
                            for oc6 in range(4):
                                oc = 4 * qi + oc6
                                hp = ps.tile([128, SQ], F32, tag="proj")
                                for dc in range(DC):
                                    nc.tensor.matmul(
                                        hp[:], w1q[:, dc, 128 * oc6:128 * (oc6 + 1)],
                                        xbf[:, dc, :], start=(dc == 0),
                                        stop=(dc == DC - 1))
                                nc.scalar.activation(htb[:, oc, :], hp[:],
                                                     AF.Gelu_apprx_tanh,
                                                     bias=b1[:, oc:oc + 1])
                        st2 = ln_stats_start()
                        for hi in range(3):
                            w2h = wf.tile([128, FC, D // 3], BF16, tag="w2h")
                            nc.sync.dma_start(
                                w2h[:], W2[l][:, :, (D // 3) * hi:(D // 3) * (hi + 1)])
                            for oc3 in range(2):
                                oc = 2 * hi + oc3
                                yp = ps.tile([128, SQ], F32, tag="proj")
                                for kc in range(FC):
                                    nc.tensor.matmul(
                                        yp[:], w2h[:, kc, 128 * oc3:128 * (oc3 + 1)],
                                        htb[:, kc, :], start=(kc == 0), stop=False)
                                nc.tensor.matmul(
                                    yp[:], b2r[:1, 128 * oc:128 * (oc + 1)],
                                    onesrow[:1, :SQ], start=False, stop=True)
                                xs = act.tile([128, SQ], BF16, tag="xs")
                                nc.vector.scalar_tensor_tensor(
                                    xs[:], xT[:, oc, :], 1.0, yp[:],
                                    op0=MUL, op1=ADD)
                                nc.vector.tensor_tensor(xT[:, oc, :], xT[:, oc, :],
                                                        yp[:], op=ADD)
                                ln_stats_chunk(st2, oc, xs[:])
                        ln_tail_apply(st2, 2 + 2 * l)
                        if l + 1 < L:
                            bvr = wp.tile([1, D], BF16, tag="bvr")
                            nc.sync.dma_start(bvr[:], BROW[l + 1][0:1, :])
                            bor = wp.tile([1, D], BF16, tag="bor")
                            nc.sync.dma_start(bor[:], BROW[l + 1][1:2, :])
                            b2r = wp.tile([1, D], BF16, tag="b2r")
                            nc.sync.dma_start(b2r[:], BROW[l + 1][2:3, :])
                            wo = wp.tile([128, DC, D], BF16, tag="wo")
                            nc.sync.dma_start(wo[:], WO[l + 1])

            with tc.tile_pool(name="fin", bufs=1) as fin, \
                 tc.tile_pool(name="fw", bufs=4) as fw, \
                 tc.tile_pool(name="fo", bufs=3) as fo, \
                 tc.tile_pool(name="fps", bufs=2, space="PSUM") as fps:
                xbf2 = fin.tile([128, DC, SQ], BF16)
                nc.vector.tensor_copy(xbf2[:], xT[:])
                nc.sync.dma_start(
                    bncx_in[:].rearrange("(p i q) -> p i q", p=128, i=DC), xbf2[:])
                nc.gpsimd.collective_compute(
                    "AllGather", mybir.AluOpType.bypass,
                    replica_groups=[list(range(NC))],
                    ins=[bncx_in[:].opt()], outs=[bncx_out[:].opt()])
                xf = fin.tile([128, DC, S], BF16)
                nc.gpsimd.dma_start(
                    xf[:].rearrange("p i (r q) -> p i r q", r=NC),
                    bncx_out[:].rearrange("(r p i q) -> p i r q",
                                          r=NC, p=128, i=DC))
                fcb = fin.tile([1, VSH], BF16)
                nc.sync.dma_start(fcb[:], FCB[:])
                for oc in range(VSH // 128):
                    fwt = fw.tile([128, DC, 128], BF16, tag="fwt")
                    nc.sync.dma_start(fwt[:], FCW[:, :, 128 * oc:128 * (oc + 1)])
                    fp = fps.tile([128, S], F32, tag="fp")
                    for ncol in range(4):
                        cs = slice(512 * ncol, 512 * (ncol + 1))
                        for dc in range(DC):
                            nc.tensor.matmul(fp[:, cs], fwt[:, dc, :],
                                             xf[:, dc, cs], start=(dc == 0),
                                             stop=False)
                        nc.tensor.matmul(fp[:, cs],
                                         fcb[0:1, 128 * oc:128 * (oc + 1)],
                                         onesrow[:1, :], start=False, stop=True)
                    ot = fo.tile([128, S], F32, tag="ot")
                    if oc % 2 == 0:
                        nc.scalar.activation(ot[:], fp[:], AF.Copy)
                    else:
                        nc.vector.tensor_copy(ot[:], fp[:])
                    nc.sync.dma_start(OUT[oc], ot[:])
    nc.finalize()
    return nc


def kernel(input_ids, attention_mask, token_type_ids, word_emb, pos_emb, type_emb,
           emb_ln_s, emb_ln_b, Wq, Wk, Wv, bq, bk, bv, Wo, bo, ln1_s, ln1_b,
           W1, b1, W2, b2, ln2_s, ln2_b, fc_w, fc_b):
    f32 = np.float32
    bf = ml_dtypes.bfloat16
    ids = np.asarray(input_ids)[0]
    tt = np.asarray(token_type_ids)[0]
    am = np.asarray(attention_mask)[0].astype(f32)
    E = (np.asarray(word_emb, f32)[ids] + np.asarray(pos_emb, f32)
         + np.asarray(type_emb, f32)[tt])

    def tp(w):  # [D, N] -> [128, DC, N] bf16
        return np.ascontiguousarray(
            np.asarray(w, f32).reshape(-1, 128, w.shape[-1]).transpose(1, 0, 2)
        ).astype(bf)

    def col(b, scale=1.0):  # [N*128] -> [128, N] f32
        b = np.asarray(b, f32) * scale
        return np.ascontiguousarray(b.reshape(-1, 128).T).astype(f32)

    wq_h = np.stack([tp(Wq[l]) for l in range(L)])
    wk_h = np.stack([tp(Wk[l]) for l in range(L)])
    wv_h = np.stack([tp(Wv[l]) for l in range(L)])
    wo_h = np.stack([tp(Wo[l]) for l in range(L)])
    w1_h = np.stack([tp(W1[l]) for l in range(L)])
    w2_h = np.stack([tp(W2[l]) for l in range(L)])
    bq_h = np.stack([col(bq[l], 0.125) for l in range(L)])
    bk_h = np.stack([col(bk[l]) for l in range(L)])
    b1_h = np.stack([col(b1[l]) for l in range(L)])
    brow_h = np.stack([np.stack([np.asarray(bv[l], f32), np.asarray(bo[l], f32),
                                 np.asarray(b2[l], f32)]) for l in range(L)]).astype(bf)

    lns_list = [np.asarray(emb_ln_s, f32)]
    lnb_list = [np.asarray(emb_ln_b, f32)]
    for l in range(L):
        lns_list += [np.asarray(ln1_s[l], f32), np.asarray(ln2_s[l], f32)]
        lnb_list += [np.asarray(ln1_b[l], f32), np.asarray(ln2_b[l], f32)]
    ln_s = np.stack([s.reshape(DC, 128).T for s in lns_list], axis=1)
    ln_b = np.stack([s.reshape(DC, 128).T for s in lnb_list], axis=1)

    blk = _block_map(NB, R)
    allowed = np.zeros((NB, NB), f32)
    for i in range(NB):
        allowed[i, blk[i]] = 1.0
    allowed[0, :] = 1.0
    allowed[NB - 1, :] = 1.0
    allow_key = np.repeat(allowed, BS, axis=1) * am[None, :]

    oneh_h = np.zeros((128, 4, HD), f32)
    for k in range(4):
        oneh_h[32 * k, k, :] = 1.0
    oneh_h = oneh_h.astype(bf)

    fcw_pad = np.zeros((D, VSH * NC), f32)
    fcw_pad[:, :V] = np.asarray(fc_w, f32)
    fcb_pad = np.zeros((VSH * NC,), f32)
    fcb_pad[:V] = np.asarray(fc_b, f32)

    if "k" not in _nc_cache:
        _nc_cache["k"] = build()
    nc = _nc_cache["k"]

    in_maps = []
    for c in range(NC):
        e_shard = E[SQ * c:SQ * (c + 1)].T.reshape(DC, 128, SQ).transpose(1, 0, 2)
        # flat mask [128, KC, SQ]: m01[p, kc, q] = allow(qblock(q), key kc*128+p)
        m01 = np.repeat(
            allow_key[4 * c:4 * c + 4].T.reshape(KC, 128, 4).transpose(1, 0, 2),
            BS, axis=2)
        in_maps.append({
            "e_t": np.ascontiguousarray(e_shard).astype(f32),
            "wq": wq_h, "wk": wk_h, "wv": wv_h, "wo": wo_h,
            "w1": w1_h, "w2": w2_h,
            "bq_t": bq_h, "bk_t": bk_h, "b1_t": b1_h, "brow": brow_h,
            "ln_s": ln_s, "ln_b": ln_b,
            "m01t": np.ascontiguousarray(m01).astype(bf),
            "oneh": oneh_h,
            "fcw": tp(fcw_pad[:, VSH * c:VSH * (c + 1)]),
            "fcb": fcb_pad[None, VSH * c:VSH * (c + 1)].astype(bf),
        })

    trace = bool(int(os.environ.get("BB_TRACE", "0")))
    res = run_bass_kernel_spmd(nc, in_maps, core_ids=list(range(NC)), trace=trace)
    kernel.last_exec_ns = getattr(res, "exec_time_ns", None)
    kernel.last_result = res
    outs = [res.results[c]["out_t"].reshape(VSH, S).T for c in range(NC)]
    logits = np.concatenate(outs, axis=1)[:, :V]
    return logits[None].astype(np.float32)


# revision 26
# speedup vs baseline: 1.1045x; 1.0315x over previous
"""BigBird encoder + vocab projection on 8 Trainium2 NeuronCores.

Sequence-sharded transformer (core c owns rows [256c, 256c+256) = 4 query
blocks), per-layer split AllGather of K^T then V (overlapped with Q proj +
attention), vocab-sharded final projection.  Activations live transposed
(xT [768part, 256free]) so every matmul consumes weights as stored.
BigBird sparsity (window + global + random + dedup + key_mask) is folded
into a per-core 0/1 mask multiplied into exp(scores) — mathematically
identical to the reference's gather+softmax.  bf16 matmuls, fp32
accumulate/residual.

Perf structure (v2): K-AG and V-AG launched as soon as each projection
finishes; copy-outs are two large p-major DMAs each; score matmuls of head
pairs interleave on PE row-groups 0-63/64-127 for 2x concurrency; softmax
reciprocals batched into one [12,SQ] op; LayerNorm stats accumulate
chunk-wise inside the residual loops to keep PE busy (HAM warm).
"""
import os, sys
os.environ.setdefault("JAX_PLATFORMS", "")
import numpy as np
import ml_dtypes

sys.path.insert(0, "/opt/trn_rl_repo")

import concourse.bass as bass
import concourse.tile as tile
from concourse import bacc, mybir
from concourse.bass_utils import run_bass_kernel_spmd

BF16 = mybir.dt.bfloat16
FP8 = mybir.dt.float8e4
F32 = mybir.dt.float32
AF = mybir.ActivationFunctionType
MUL = mybir.AluOpType.mult
ADD = mybir.AluOpType.add
SUB = mybir.AluOpType.subtract

B, S, D, F, V = 1, 2048, 768, 3072, 50358
L, H, HD, BS, NB, R = 12, 12, 64, 64, 32, 3
NC = 8
SQ = S // NC                # 256
DC = D // 128               # 6
FC = F // 128               # 24
KC = S // 128               # 16
VSH = 6400                  # padded vocab shard (50 x 128)
VN_E = HD + 1               # 65 cols per head in V-normal (ones col for rowsum)
VNF = H * VN_E              # 780 flat per (p, rc)
VN_FLAT = 128 * 2 * VNF     # 199680
KT_FLAT = 128 * DC * SQ     # 196608
XF_FLAT = 128 * DC * SQ

_nc_cache = {}


def _block_map(nb, r, seed=0):
    rng = np.random.default_rng(seed)
    idx = np.zeros((nb, 5 + r), np.int32)
    for i in range(nb):
        lst = [0, nb - 1, max(i - 1, 0), i, min(i + 1, nb - 1)]
        cand = np.setdiff1d(np.arange(nb), np.array(lst))
        lst += list(rng.choice(cand, r, replace=False))
        for j, b in enumerate(lst):
            idx[i, j] = int(b)
    return idx


def build():
    nc = bacc.Bacc("TRN2", target_bir_lowering=False, debug=False, num_devices=NC)
    ET = nc.dram_tensor("e_t", [128, DC, SQ], F32, kind="ExternalInput")
    WQ = nc.dram_tensor("wq", [L, 128, DC, D], BF16, kind="ExternalInput")
    WK = nc.dram_tensor("wk", [L, 128, DC, D], BF16, kind="ExternalInput")
    WV = nc.dram_tensor("wv", [L, 128, DC, D], BF16, kind="ExternalInput")
    WO = nc.dram_tensor("wo", [L, 128, DC, D], BF16, kind="ExternalInput")
    W1 = nc.dram_tensor("w1", [L, 128, DC, F], BF16, kind="ExternalInput")
    W2 = nc.dram_tensor("w2", [L, 128, FC, D], BF16, kind="ExternalInput")
    BQ = nc.dram_tensor("bq_t", [L, 128, DC], F32, kind="ExternalInput")  # x0.125
    BK = nc.dram_tensor("bk_t", [L, 128, DC], F32, kind="ExternalInput")
    B1 = nc.dram_tensor("b1_t", [L, 128, FC], F32, kind="ExternalInput")
    BROW = nc.dram_tensor("brow", [L, 3, D], BF16, kind="ExternalInput")  # bv,bo,b2
    LNS = nc.dram_tensor("ln_s", [128, 2 * L + 1, DC], F32, kind="ExternalInput")
    LNB = nc.dram_tensor("ln_b", [128, 2 * L + 1, DC], F32, kind="ExternalInput")
    M01 = nc.dram_tensor("m01t", [128, KC, SQ], BF16, kind="ExternalInput")
    ONEH = nc.dram_tensor("oneh", [128, 4, HD], BF16, kind="ExternalInput")
    FCW = nc.dram_tensor("fcw", [128, DC, VSH], BF16, kind="ExternalInput")
    FCB = nc.dram_tensor("fcb", [128, VSH // 128], F32, kind="ExternalInput")
    OUT = nc.dram_tensor("out_t", [VSH // 128, 128, S], F32, kind="ExternalOutput")

    with tile.TileContext(nc) as tc:
        with tc.tile_pool(name="dram", bufs=1, space="DRAM") as dram, \
             tc.tile_pool(name="res", bufs=1) as res, \
             tc.tile_pool(name="const", bufs=1) as const:
            AGF = KT_FLAT + VN_FLAT
            bnc_ins = [dram.tile([AGF], FP8, tag=f"cki{l}", name=f"cki{l}")
                       for l in range(L)]
            bnc_outs = [dram.tile([NC * AGF], FP8, addr_space="Shared",
                                  tag=f"cko{l}", name=f"cko{l}")
                        for l in range(L)]
            bncx_in = dram.tile([XF_FLAT], BF16)
            bncx_out = dram.tile([NC * XF_FLAT], BF16, addr_space="Shared")

            xT = res.tile([128, DC, SQ], F32)
            nc.sync.dma_start(xT[:], ET[:])
            m01 = const.tile([128, KC, SQ], BF16)
            nc.sync.dma_start(m01[:], M01[:])
            oneh = const.tile([128, 4, HD], BF16)
            nc.sync.dma_start(oneh[:], ONEH[:])
            lns = const.tile([128, 2 * L + 1, DC], F32)
            lnb = const.tile([128, 2 * L + 1, DC], F32)
            nc.sync.dma_start(lns[:], LNS[:])
            nc.sync.dma_start(lnb[:], LNB[:])
            ones_b = const.tile([128, 1], BF16)
            nc.vector.memset(ones_b[:], 1.0)
            ones_fr = const.tile([1, 128], F32)
            nc.vector.memset(ones_fr[:], 1.0)
            onesrow = const.tile([1, 512], BF16)
            nc.vector.memset(onesrow[:], 1.0)
            eps = const.tile([1, 1], F32)
            nc.vector.memset(eps[:], 1e-12)

            with tc.tile_pool(name="wp", bufs=1) as wp, \
                 tc.tile_pool(name="wf", bufs=2) as wf, \
                 tc.tile_pool(name="act", bufs=2) as act, \
                 tc.tile_pool(name="ag", bufs=1) as ag, \
                 tc.tile_pool(name="sm", bufs=2) as sm, \
                 tc.tile_pool(name="ps", bufs=2, space="PSUM") as ps, \
                 tc.tile_pool(name="pssc", bufs=2, space="PSUM") as pssc, \
                 tc.tile_pool(name="ps1", bufs=2, space="PSUM") as ps1:

                xbf = act.tile([128, DC, SQ], BF16, tag="xbf", bufs=1)
                # softmax denominator parking (head h -> partition 32*(h%4),
                # group h//4); memset once so full-tile reciprocal never sees
                # uninitialized bits
                dnt = sm.tile([128, 3, SQ], F32, tag="dnt", bufs=1)
                nc.vector.memset(dnt[:], 1.0)

                # ---- layernorm helpers (chunk-pipelined) ----
                def ln_stats_start():
                    su = pssc.tile([1, SQ], F32, tag="sc", name="lnsum")
                    sq = pssc.tile([1, SQ], F32, tag="sc", name="lnssq")
                    return su, sq

                def ln_stats_chunk(stats, oc, src_bf):
                    su, sq = stats
                    sqs = act.tile([128, SQ], BF16, tag="sqs")
                    nc.scalar.activation(sqs[:], src_bf, AF.Square)
                    nc.tensor.matmul(su[:], ones_b[:], src_bf,
                                     start=(oc == 0), stop=(oc == DC - 1))
                    nc.tensor.matmul(sq[:], ones_b[:], sqs[:],
                                     start=(oc == 0), stop=(oc == DC - 1))

                def ln_tail_apply(stats, li):
                    su, sq = stats
                    nmean = sm.tile([1, SQ], F32, tag="nmean")
                    ab = sm.tile([1, 2 * SQ], F32, tag="ab")
                    nc.vector.tensor_scalar_mul(nmean[:], su[:], -1.0 / D)
                    nc.vector.tensor_scalar_mul(ab[:, 0:SQ], sq[:], 1.0 / D)
                    t0 = sm.tile([1, SQ], F32, tag="lnt0")
                    nc.vector.tensor_tensor(t0[:], nmean[:], nmean[:], op=MUL)
                    nc.vector.tensor_tensor(ab[:, 0:SQ], ab[:, 0:SQ], t0[:], op=SUB)
                    # rstd = 1/sqrt(var + eps)
                    nc.scalar.activation(t0[:], ab[:, 0:SQ], AF.Sqrt, bias=eps[:])
                    nc.vector.reciprocal_approx_fast(ab[:, 0:SQ], t0[:])
                    nc.vector.tensor_tensor(ab[:, SQ:], nmean[:], ab[:, 0:SQ],
                                            op=MUL)              # -mean*rstd
                    bc = pssc.tile([128, 2 * SQ], F32, tag="sc", name="lnbc")
                    nc.tensor.matmul(bc[:], ones_fr[:], ab[:], start=True, stop=True)
                    # xbf (bf16, consumed by matmuls) is produced with the
                    # shortest possible vector chain; the fp32 xT update (+b)
                    # is deferred so it doesn't gate the next phase's PE work
                    t2s = []
                    for oc in range(DC):
                        t1 = sm.tile([128, SQ], F32, tag="lnt1")
                        nc.vector.scalar_tensor_tensor(
                            t1[:], xT[:, oc, :], lns[:, li, oc:oc + 1],
                            bc[:, 0:SQ], op0=MUL, op1=MUL)
                        t2 = sm.tile([128, SQ], F32, tag="lnt2", bufs=6)
                        nc.vector.scalar_tensor_tensor(
                            t2[:], bc[:, SQ:2 * SQ], lns[:, li, oc:oc + 1],
                            t1[:], op0=MUL, op1=ADD)
                        nc.scalar.activation(xbf[:, oc, :], t2[:], AF.Identity,
                                             bias=lnb[:, li, oc:oc + 1])
                        t2s.append(t2)
                    for oc in range(DC):
                        nc.vector.tensor_scalar_add(xT[:, oc, :], t2s[oc][:],
                                                    lnb[:, li, oc:oc + 1])

                # ---- embedding layernorm (standalone) ----
                wk = wp.tile([128, DC, D], BF16, tag="wk")
                nc.sync.dma_start(wk[:], WK[0])
                wv = wp.tile([128, DC, D], BF16, tag="wv")
                nc.sync.dma_start(wv[:], WV[0])
                wq = wp.tile([128, DC, D], BF16, tag="wq")
                nc.sync.dma_start(wq[:], WQ[0])
                bq = wp.tile([128, DC], F32, tag="bq")
                nc.sync.dma_start(bq[:], BQ[0])
                bk = wp.tile([128, DC], F32, tag="bk")
                nc.sync.dma_start(bk[:], BK[0])
                bvr = wp.tile([1, D], BF16, tag="bvr")
                nc.sync.dma_start(bvr[:], BROW[0][0:1, :])
                bor = wp.tile([1, D], BF16, tag="bor")
                nc.sync.dma_start(bor[:], BROW[0][1:2, :])
                b2r = wp.tile([1, D], BF16, tag="b2r")
                nc.sync.dma_start(b2r[:], BROW[0][2:3, :])
                wo = wp.tile([128, DC, D], BF16, tag="wo")
                nc.sync.dma_start(wo[:], WO[0])

                st0 = ln_stats_start()
                for oc in range(DC):
                    xs = act.tile([128, SQ], BF16, tag="xs")
                    nc.vector.tensor_copy(xs[:], xT[:, oc, :])
                    ln_stats_chunk(st0, oc, xs[:])
                ln_tail_apply(st0, 0)

                for l in range(L):
                    with nc.named_scope(f"L{l}"):
                        # ---- K^T projection + AG launch ----
                        ktb = act.tile([128, DC, SQ], FP8, tag="ktb", bufs=1)
                        for oc in range(DC):
                            kp = ps.tile([128, SQ], F32, tag="proj")
                            for dc in range(DC):
                                nc.tensor.matmul(
                                    kp[:], wk[:, dc, 128 * oc:128 * (oc + 1)],
                                    xbf[:, dc, :], start=(dc == 0),
                                    stop=(dc == DC - 1))
                            nc.scalar.activation(ktb[:, oc, :], kp[:],
                                                 AF.Identity,
                                                 bias=bk[:, oc:oc + 1])
                        nc.sync.dma_start(
                            bnc_ins[l][0:KT_FLAT].rearrange("(p i q) -> p i q",
                                                            p=128, i=DC), ktb[:])

                        # ---- V normal + AG launch ----
                        vnb = act.tile([128, 2, VNF], FP8, tag="vnb", bufs=1)
                        nc.vector.memset(
                            vnb[:].rearrange("p rc (h e) -> p rc h e",
                                             h=H)[:, :, :, HD:], 1.0)
                        for rc in range(2):
                            for hf in range(2):
                                cs = slice(384 * hf, 384 * (hf + 1))
                                vp = ps.tile([128, 384], F32, tag="proj",
                                             name="vproj")
                                for dc in range(DC):
                                    nc.tensor.matmul(
                                        vp[:], xbf[:, dc, 128 * rc:128 * (rc + 1)],
                                        wv[:, dc, cs], start=(dc == 0), stop=False)
                                nc.tensor.matmul(vp[:], onesrow[:1, :128],
                                                 bvr[:1, cs], start=False, stop=True)
                                nc.vector.tensor_copy(
                                    vnb[:, rc, 390 * hf:390 * (hf + 1)].rearrange(
                                        "p (h e) -> p h e", h=6)[:, :, 0:HD],
                                    vp[:].rearrange("p (h e) -> p h e", h=6))
                        nc.sync.dma_start(
                            bnc_ins[l][KT_FLAT:].rearrange("(p rc f) -> p rc f",
                                                           p=128, rc=2), vnb[:])
                        nc.gpsimd.collective_compute(
                            "AllGather", mybir.AluOpType.bypass,
                            replica_groups=[list(range(NC))],
                            ins=[bnc_ins[l][:].opt()],
                            outs=[bnc_outs[l][:].opt()])

                        # ---- Q^T (overlaps AG) ----
                        qtb = act.tile([128, DC, SQ], BF16, tag="qtb", bufs=1)
                        for oc in range(DC):
                            qp = ps.tile([128, SQ], F32, tag="proj")
                            for dc in range(DC):
                                nc.tensor.matmul(
                                    qp[:], wq[:, dc, 128 * oc:128 * (oc + 1)],
                                    xbf[:, dc, :], start=(dc == 0),
                                    stop=(dc == DC - 1))
                            nc.scalar.activation(qtb[:, oc, :], qp[:],
                                                 AF.Identity,
                                                 bias=bq[:, oc:oc + 1],
                                                 scale=0.125)

                        # prefetch next layer's QKV weights
                        if l + 1 < L:
                            wk = wp.tile([128, DC, D], BF16, tag="wk")
                            nc.sync.dma_start(wk[:], WK[l + 1])
                            wv = wp.tile([128, DC, D], BF16, tag="wv")
                            nc.sync.dma_start(wv[:], WV[l + 1])
                            wq = wp.tile([128, DC, D], BF16, tag="wq")
                            nc.sync.dma_start(wq[:], WQ[l + 1])
                            bq = wp.tile([128, DC], F32, tag="bq")
                            nc.sync.dma_start(bq[:], BQ[l + 1])
                            bk = wp.tile([128, DC], F32, tag="bk")
                            nc.sync.dma_start(bk[:], BK[l + 1])

                        # ---- AG copy-outs (2 halves each, gpsimd queue) ----
                        ktag = ag.tile([128, NC * DC, SQ], FP8, tag="ktag")
                        vnag = ag.tile([128, KC, VNF], FP8, tag="vnag")
                        bout = bnc_outs[l][:].rearrange("(r f) -> r f", r=NC)
                        for hh, eng in ((0, nc.sync), (1, nc.gpsimd)):
                            eng.dma_start(
                                ktag[:, 24 * hh:24 * (hh + 1), :].rearrange(
                                    "p (r i) q -> p r i q", r=4),
                                bout[4 * hh:4 * (hh + 1), 0:KT_FLAT].rearrange(
                                    "r (p i q) -> p r i q", p=128, i=DC))
                        for hh, eng in ((0, nc.sync), (1, nc.gpsimd)):
                            eng.dma_start(
                                vnag[:, 8 * hh:8 * (hh + 1), :].rearrange(
                                    "p (r rc) f -> p r rc f", r=4),
                                bout[4 * hh:4 * (hh + 1), KT_FLAT:].rearrange(
                                    "r (p rc f) -> p r rc f", p=128, rc=2))

                        # ---- attention: head pairs, interleaved row groups ----
                        ctxr = act.tile([128, DC, SQ], BF16, tag="ctxr", bufs=1)
                        ctxb = act.tile([128, DC, SQ], BF16, tag="ctxb", bufs=1)
                        for t in range(6):
                            hA, hB = 2 * t, 2 * t + 1
                            ctA = ps1.tile([VN_E, SQ], F32, tag="ctx")
                            ctB = ps1.tile([VN_E, SQ], F32, tag="ctx")
                            for w in range(4):
                                scA = pssc.tile([128, 4, SQ], F32, tag="sc")
                                scB = pssc.tile([128, 4, SQ], F32, tag="sc")
                                for j in range(4):
                                    kc = 4 * w + j
                                    r, hf = kc // 2, kc % 2
                                    nc.tensor.matmul(
                                        scA[:, j, :],
                                        ktag[0:64, DC * r + t,
                                             128 * hf:128 * (hf + 1)],
                                        qtb[0:64, t, :], start=True, stop=True)
                                    nc.tensor.matmul(
                                        scB[:, j, :],
                                        ktag[64:128, DC * r + t,
                                             128 * hf:128 * (hf + 1)],
                                        qtb[64:128, t, :], start=True, stop=True)
                                prA = sm.tile([128, 4, SQ], BF16, tag="pr",
                                              bufs=8)
                                prB = sm.tile([128, 4, SQ], BF16, tag="pr",
                                              bufs=8)
                                nc.scalar.activation(prA[:], scA[:], AF.Exp)
                                nc.scalar.activation(prB[:], scB[:], AF.Exp)
                                nc.vector.tensor_tensor(
                                    prA[:], prA[:], m01[:, 4 * w:4 * w + 4, :],
                                    op=MUL)
                                nc.vector.tensor_tensor(
                                    prB[:], prB[:], m01[:, 4 * w:4 * w + 4, :],
                                    op=MUL)
                                for j in range(4):
                                    kc = 4 * w + j
                                    nc.tensor.matmul(
                                        ctA[:], vnag[:, kc, 65 * hA:65 * hA + 65],
                                        prA[:, j, :], start=(kc == 0),
                                        stop=(kc == KC - 1))
                                    nc.tensor.matmul(
                                        ctB[:], vnag[:, kc, 65 * hB:65 * hB + 65],
                                        prB[:, j, :], start=(kc == 0),
                                        stop=(kc == KC - 1))
                            pA, gA = 32 * (hA % 4), hA // 4
                            pB, gB = 32 * (hB % 4), hB // 4
                            nc.vector.tensor_copy(dnt[pA:pA + 1, gA, :],
                                                  ctA[HD:HD + 1, :])
                            nc.vector.tensor_copy(dnt[pB:pB + 1, gB, :],
                                                  ctB[HD:HD + 1, :])
                            nc.vector.tensor_copy(ctxr[0:64, t, :], ctA[0:HD, :])
                            nc.vector.tensor_copy(ctxr[64:128, t, :], ctB[0:HD, :])
                        rcpf = sm.tile([128, 3, SQ], F32, tag="rcpf", bufs=1)
                        nc.vector.reciprocal_approx_fast(rcpf[:], dnt[:])
                        rcp = sm.tile([128, 3, SQ], BF16, tag="rcp", bufs=1)
                        nc.vector.tensor_copy(rcp[:], rcpf[:])
                        for h in range(H):
                            ki, kp_ = h // 2, 64 * (h % 2)
                            rbc = ps.tile([64, SQ], F32, tag="proj", name="rbc")
                            nc.tensor.matmul(rbc[:], oneh[:, h % 4, :],
                                             rcp[:, h // 4, :],
                                             start=True, stop=True)
                            nc.vector.tensor_tensor(ctxb[kp_:kp_ + 64, ki, :],
                                                    ctxr[kp_:kp_ + 64, ki, :],
                                                    rbc[:], op=MUL)

                        # ---- Wo + residual + LN1 stats (chunked) ----
                        st1 = ln_stats_start()
                        for oc in range(DC):
                            op_ = ps.tile([128, SQ], F32, tag="proj")
                            for dc in range(DC):
                                nc.tensor.matmul(
                                    op_[:], wo[:, dc, 128 * oc:128 * (oc + 1)],
                                    ctxb[:, dc, :], start=(dc == 0), stop=False)
                            nc.tensor.matmul(op_[:],
                                             bor[:1, 128 * oc:128 * (oc + 1)],
                                             onesrow[:1, :SQ], start=False,
                                             stop=True)
                            xs = act.tile([128, SQ], BF16, tag="xs")
                            nc.vector.scalar_tensor_tensor(
                                xs[:], xT[:, oc, :], 1.0, op_[:],
                                op0=MUL, op1=ADD)
                            nc.vector.tensor_tensor(xT[:, oc, :], xT[:, oc, :],
                                                    op_[:], op=ADD)
                            ln_stats_chunk(st1, oc, xs[:])
                        ln_tail_apply(st1, 1 + 2 * l)

                        # ---- FFN ----
                        b1 = wp.tile([128, FC], F32, tag="b1")
                        nc.sync.dma_start(b1[:], B1[l])
                        htb = act.tile([128, FC, SQ], BF16, tag="htb", bufs=1)
                        for qi in range(6):
                            w1q = wf.tile([128, DC, F // 6], BF16, tag="w1q")
                            nc.sync.dma_start(
                                w1q[:], W1[l][:, :, (F // 6) * qi:(F // 6) * (qi + 1)])
                            for oc6 in range(4):
                                oc = 4 * qi + oc6
                                hp = ps.tile([128, SQ], F32, tag="proj")
                                for dc in range(DC):
                                    nc.tensor.matmul(
                                        hp[:], w1q[:, dc, 128 * oc6:128 * (oc6 + 1)],
                                        xbf[:, dc, :], start=(dc == 0),
                                        stop=(dc == DC - 1))
                                nc.scalar.activation(htb[:, oc, :], hp[:],
                                                     AF.Gelu_apprx_tanh,
                                                     bias=b1[:, oc:oc + 1])
                        st2 = ln_stats_start()
                        for hi in range(3):
                            w2h = wf.tile([128, FC, D // 3], BF16, tag="w2h")
                            nc.sync.dma_start(
                                w2h[:], W2[l][:, :, (D // 3) * hi:(D // 3) * (hi + 1)])
                            for oc3 in range(2):
                                oc = 2 * hi + oc3
                                yp = ps.tile([128, SQ], F32, tag="proj")
                                for kc in range(FC):
                                    nc.tensor.matmul(
                                        yp[:], w2h[:, kc, 128 * oc3:128 * (oc3 + 1)],
                                        htb[:, kc, :], start=(kc == 0), stop=False)
                                nc.tensor.matmul(
                                    yp[:], b2r[:1, 128 * oc:128 * (oc + 1)],
                                    onesrow[:1, :SQ], start=False, stop=True)
                                xs = act.tile([128, SQ], BF16, tag="xs")
                                nc.vector.scalar_tensor_tensor(
                                    xs[:], xT[:, oc, :], 1.0, yp[:],
                                    op0=MUL, op1=ADD)
                                nc.vector.tensor_tensor(xT[:, oc, :], xT[:, oc, :],
                                                        yp[:], op=ADD)
                                ln_stats_chunk(st2, oc, xs[:])
                        ln_tail_apply(st2, 2 + 2 * l)
                        if l + 1 < L:
                            bvr = wp.tile([1, D], BF16, tag="bvr")
                            nc.sync.dma_start(bvr[:], BROW[l + 1][0:1, :])
                            bor = wp.tile([1, D], BF16, tag="bor")
                            nc.sync.dma_start(bor[:], BROW[l + 1][1:2, :])
                            b2r = wp.tile([1, D], BF16, tag="b2r")
                            nc.sync.dma_start(b2r[:], BROW[l + 1][2:3, :])
                            wo = wp.tile([128, DC, D], BF16, tag="wo")
                            nc.sync.dma_start(wo[:], WO[l + 1])

            with tc.tile_pool(name="fin", bufs=1) as fin, \
                 tc.tile_pool(name="fw", bufs=4) as fw, \
                 tc.tile_pool(name="fo", bufs=3) as fo, \
                 tc.tile_pool(name="fps", bufs=2, space="PSUM") as fps:
                xbf2 = fin.tile([128, DC, SQ], BF16)
                nc.vector.tensor_copy(xbf2[:], xT[:])
                nc.sync.dma_start(
                    bncx_in[:].rearrange("(p i q) -> p i q", p=128, i=DC), xbf2[:])
                nc.gpsimd.collective_compute(
                    "AllGather", mybir.AluOpType.bypass,
                    replica_groups=[list(range(NC))],
                    ins=[bncx_in[:].opt()], outs=[bncx_out[:].opt()])
                xf = fin.tile([128, DC, S], BF16)
                nc.gpsimd.dma_start(
                    xf[:].rearrange("p i (r q) -> p i r q", r=NC),
                    bncx_out[:].rearrange("(r p i q) -> p i r q",
                                          r=NC, p=128, i=DC))
                fcb = fin.tile([128, VSH // 128], F32)
                nc.sync.dma_start(fcb[:], FCB[:])
                for oc in range(VSH // 128):
                    fwt = fw.tile([128, DC, 128], BF16, tag="fwt")
                    nc.sync.dma_start(fwt[:], FCW[:, :, 128 * oc:128 * (oc + 1)])
                    fp = fps.tile([128, S], F32, tag="fp")
                    for ncol in range(4):
                        cs = slice(512 * ncol, 512 * (ncol + 1))
                        for dc in range(DC):
                            nc.tensor.matmul(fp[:, cs], fwt[:, dc, :],
                                             xf[:, dc, cs], start=(dc == 0),
                                             stop=(dc == DC - 1))
                    ot = fo.tile([128, S], F32, tag="ot")
                    if oc % 2 == 0:
                        nc.scalar.activation(ot[:], fp[:], AF.Identity,
                                             bias=fcb[:, oc:oc + 1])
                    else:
                        nc.vector.tensor_scalar_add(ot[:], fp[:],
                                                    fcb[:, oc:oc + 1])
                    nc.sync.dma_start(OUT[oc], ot[:])
    nc.finalize()
    return nc


def kernel(input_ids, attention_mask, token_type_ids, word_emb, pos_emb, type_emb,
           emb_ln_s, emb_ln_b, Wq, Wk, Wv, bq, bk, bv, Wo, bo, ln1_s, ln1_b,
           W1, b1, W2, b2, ln2_s, ln2_b, fc_w, fc_b):
    f32 = np.float32
    bf = ml_dtypes.bfloat16
    ids = np.asarray(input_ids)[0]
    tt = np.asarray(token_type_ids)[0]
    am = np.asarray(attention_mask)[0].astype(f32)
    E = (np.asarray(word_emb, f32)[ids] + np.asarray(pos_emb, f32)
         + np.asarray(type_emb, f32)[tt])

    def tp(w):  # [D, N] -> [128, DC, N] bf16
        return np.ascontiguousarray(
            np.asarray(w, f32).reshape(-1, 128, w.shape[-1]).transpose(1, 0, 2)
        ).astype(bf)

    def col(b, scale=1.0):  # [N*128] -> [128, N] f32
        b = np.asarray(b, f32) * scale
        return np.ascontiguousarray(b.reshape(-1, 128).T).astype(f32)

    wq_h = np.stack([tp(Wq[l]) for l in range(L)])
    wk_h = np.stack([tp(Wk[l]) for l in range(L)])
    wv_h = np.stack([tp(Wv[l]) for l in range(L)])
    wo_h = np.stack([tp(Wo[l]) for l in range(L)])
    w1_h = np.stack([tp(W1[l]) for l in range(L)])
    w2_h = np.stack([tp(W2[l]) for l in range(L)])
    bq_h = np.stack([col(bq[l], 0.125) for l in range(L)])
    bk_h = np.stack([col(bk[l]) for l in range(L)])
    b1_h = np.stack([col(b1[l]) for l in range(L)])
    brow_h = np.stack([np.stack([np.asarray(bv[l], f32), np.asarray(bo[l], f32),
                                 np.asarray(b2[l], f32)]) for l in range(L)]).astype(bf)

    lns_list = [np.asarray(emb_ln_s, f32)]
    lnb_list = [np.asarray(emb_ln_b, f32)]
    for l in range(L):
        lns_list += [np.asarray(ln1_s[l], f32), np.asarray(ln2_s[l], f32)]
        lnb_list += [np.asarray(ln1_b[l], f32), np.asarray(ln2_b[l], f32)]
    ln_s = np.stack([s.reshape(DC, 128).T for s in lns_list], axis=1)
    ln_b = np.stack([s.reshape(DC, 128).T for s in lnb_list], axis=1)

    blk = _block_map(NB, R)
    allowed = np.zeros((NB, NB), f32)
    for i in range(NB):
        allowed[i, blk[i]] = 1.0
    allowed[0, :] = 1.0
    allowed[NB - 1, :] = 1.0
    allow_key = np.repeat(allowed, BS, axis=1) * am[None, :]

    oneh_h = np.zeros((128, 4, HD), f32)
    for k in range(4):
        oneh_h[32 * k, k, :] = 1.0
    oneh_h = oneh_h.astype(bf)

    fcw_pad = np.zeros((D, VSH * NC), f32)
    fcw_pad[:, :V] = np.asarray(fc_w, f32)
    fcb_pad = np.zeros((VSH * NC,), f32)
    fcb_pad[:V] = np.asarray(fc_b, f32)

    if "k" not in _nc_cache:
        _nc_cache["k"] = build()
    nc = _nc_cache["k"]

    in_maps = []
    for c in range(NC):
        e_shard = E[SQ * c:SQ * (c + 1)].T.reshape(DC, 128, SQ).transpose(1, 0, 2)
        # flat mask [128, KC, SQ]: m01[p, kc, q] = allow(qblock(q), key kc*128+p)
        m01 = np.repeat(
            allow_key[4 * c:4 * c + 4].T.reshape(KC, 128, 4).transpose(1, 0, 2),
            BS, axis=2)
        in_maps.append({
            "e_t": np.ascontiguousarray(e_shard).astype(f32),
            "wq": wq_h, "wk": wk_h, "wv": wv_h, "wo": wo_h,
            "w1": w1_h, "w2": w2_h,
            "bq_t": bq_h, "bk_t": bk_h, "b1_t": b1_h, "brow": brow_h,
            "ln_s": ln_s, "ln_b": ln_b,
            "m01t": np.ascontiguousarray(m01).astype(bf),
            "oneh": oneh_h,
            "fcw": tp(fcw_pad[:, VSH * c:VSH * (c + 1)]),
            "fcb": np.ascontiguousarray(
                fcb_pad[VSH * c:VSH * (c + 1)].reshape(VSH // 128, 128).T
            ).astype(f32),
        })

    trace = bool(int(os.environ.get("BB_TRACE", "0")))
    res = run_bass_kernel_spmd(nc, in_maps, core_ids=list(range(NC)), trace=trace)
    kernel.last_exec_ns = getattr(res, "exec_time_ns", None)
    kernel.last_result = res
    outs = [res.results[c]["out_t"].reshape(VSH, S).T for c in range(NC)]
    logits = np.concatenate(outs, axis=1)[:, :V]
    return logits[None].astype(np.float32)
